# revision 57
# baseline (speedup 1.0000x reference)
"""RWKV-4 block (TimeMix + ChannelMix) Trainium2 Bass kernel.

Full inputs in, full outputs out. Sharding: data-parallel over batch B=8
across the 8 NeuronCores (one batch element per core, no collectives).

Per-core layout strategy:
  - LayerNorm + stats in token-major [128 tokens, C] (bn_stats along free
    dim), then XBAR DMA-transpose to feature-major [128 chan, T] in one op
    per token tile. Channels are interleaved across partitions
    (channel c = p*8 + cb lives at [partition p, block cb]) so the XBAR's
    row-write order matches the [p, cb, t] tile layout; weight rows and
    per-channel mix params are permuted host-side to match.
  - token_shift is a free-dim offset view of a front-padded feature-major
    tile; the pad column is copied from the previous chunk's last column.
  - k/v/r and FFN matmuls in bf16 (fp32 PSUM accumulation).
  - WKV recurrence A_t = lam*A_{t-1} + e^{k_t+u} v_t (and B with e^{k_t+u})
    runs as a native DVE tensor_tensor_scan per 128-channel block, fp32,
    chained across T-chunks via the `initial` operand (prev scan's last
    element). e^u is folded into the exp bias host-side: it cancels in
    y = num/den, so num = ekv + A_prev and den = ek + B_prev are plain adds.
    Division via ACT Ln+Exp (1/x = exp(-ln x), denominator positive).
  - Elementwise work is spread across engines: DVE (scans, adds, k-mix),
    GpSimd (v/r mixes, subs, rw mult), ACT (exp/ln, psv+bv via Copy bias).
"""

import os
import sys

import numpy as np

if "/opt/trn_rl_repo" not in sys.path:
    sys.path.insert(0, "/opt/trn_rl_repo")

import ml_dtypes

B, T, C, FFN = 8, 2048, 1024, 4096
NCORES = 8
CB = C // 128      # 8 channel blocks
FB = FFN // 128    # 32 ffn blocks
TCA = 512          # TimeMix chunk (tokens)
NCHA = T // TCA    # 4
TCB = 256          # FFN chunk (tokens)
NCHB = T // TCB    # 8
LN_EPS = 1e-5

_CACHE = {}

# pars packing: 10 per-C params as [128, 8] panels + bkf as [128, 32]
PAR_NAMES = ["mk", "mv", "mr", "mfk", "mfr", "lam", "bk", "bku", "bv", "brn"]
NPARC = len(PAR_NAMES) * CB + FB  # 112


def _build_program(with_b2r=False):
    import concourse.bass as bass  # noqa: F401
    from concourse import bacc
    import concourse.tile as tile
    import concourse.mybir as mybir
    from contextlib import ExitStack

    f32 = mybir.dt.float32
    bf16 = mybir.dt.bfloat16
    AF = mybir.ActivationFunctionType
    OP = mybir.AluOpType

    # Pin ALL activations to the one table set that contains every function
    # we use (ln/exp/relu/copy). Without this the table-load pass picks the
    # first set containing each function and thrashes ACT_TABLE_LOADs
    # (~2.7us each).
    import concourse.hw_specs as hw_specs
    if not getattr(hw_specs, "_rwkv_patched", False):
        _orig_gat = hw_specs.get_activation_tables

        def _only_lnexp(arch):
            t = _orig_gat(arch)
            keep = "natural_log_exp_and_others"
            return {name: (funcs if name == keep else set())
                    for name, funcs in t.items()}

        hw_specs.get_activation_tables = _only_lnexp
        bacc.get_activation_tables = _only_lnexp
        hw_specs._rwkv_patched = True

    nc = bacc.Bacc("TRN2", target_bir_lowering=False, debug=False,
                   enable_asserts=False)

    # ---------------- DRAM I/O ----------------
    x_d = nc.dram_tensor("x", [T, C], bf16, kind="ExternalInput").ap()
    y_d = nc.dram_tensor("y", [T, C], f32, kind="ExternalOutput").ap()

    # Weights are pre-arranged host-side to the SBUF tile layout
    # [128, blocks, cols] (partition-contiguous) so each weight DMA needs
    # only one descriptor per partition — descriptor generation for the
    # naive strided layout serialized the SP queue for ~6us per transfer.
    wk_d = nc.dram_tensor("wkT", [128, CB, C], bf16, kind="ExternalInput").ap()
    wv_d = nc.dram_tensor("wvT", [128, CB, C], bf16, kind="ExternalInput").ap()
    wr_d = nc.dram_tensor("wrT", [128, CB, C], bf16, kind="ExternalInput").ap()
    wo_d = nc.dram_tensor("woT", [128, CB, C], bf16, kind="ExternalInput").ap()
    wkf_d = nc.dram_tensor("wkfT", [128, 4, CB, FFN // 4], bf16,
                           kind="ExternalInput").ap()
    wvf_d = nc.dram_tensor("wvfT", [128, FB, C], bf16, kind="ExternalInput").ap()
    wrf_d = nc.dram_tensor("wrfT", [128, CB, C], bf16, kind="ExternalInput").ap()

    # All small per-channel params packed into ONE [128, NPARC] fp32 tensor
    # (a single contiguous DMA instead of ~11 tiny strided ones).
    pars_d = nc.dram_tensor("pars", [128, NPARC], f32, kind="ExternalInput").ap()
    if with_b2r:
        b2r_d = nc.dram_tensor("b2r", [C], f32, kind="ExternalInput").ap()

    with tile.TileContext(nc) as tc, ExitStack() as glob:
        const = glob.enter_context(tc.tile_pool(name="const", bufs=1))
        dram = glob.enter_context(tc.tile_pool(name="dram", bufs=1, space="DRAM"))

        pars_t = const.tile([128, NPARC], f32)
        par = {}
        for i, p in enumerate(PAR_NAMES):
            par[p] = pars_t[:, i * CB:(i + 1) * CB]
        bkf_t = pars_t[:, len(PAR_NAMES) * CB:len(PAR_NAMES) * CB + FB]
        if with_b2r:
            b2r_t = const.tile([128, C], f32)

        eps_t = const.tile([128, 1], f32)
        one_t = const.tile([128, 1], f32)

        # x2 is split per B-chunk so phase B's reads only depend on the
        # A-chunk that produced those tokens (whole-tile DRAM deps would
        # stall B0 until all of A finished).
        x2_dram = [dram.tile([TCB, C], bf16, name=f"x2d{j}")
                   for j in range(NCHB)]

        # ---- helper: LN stats + apply for one [128, C] token tile ----
        def ln_apply(xt, statsp, xnp):
            st6 = statsp.tile([128, 2, 6], f32, name="st6", tag="st6")
            nc.vector.bn_stats(out=st6[:, 0, :], in_=xt[:, 0:512])
            nc.vector.bn_stats(out=st6[:, 1, :], in_=xt[:, 512:1024])
            mv_t = statsp.tile([128, 2], f32, name="mv_t", tag="mv")
            nc.vector.bn_aggr(out=mv_t, in_=st6)
            # rstd = exp(-0.5*ln(var+eps)) — keeps ACT inside the ln/exp
            # table set (Sqrt lives in a different set -> ~2.7us reload).
            ldv = statsp.tile([128, 1], f32, name="ldv", tag="ldv")
            nc.scalar.activation(out=ldv, in_=mv_t[:, 1:2], func=AF.Ln,
                                 bias=eps_t)
            rstd = statsp.tile([128, 1], f32, name="rstd", tag="rstd")
            nc.scalar.activation(out=rstd, in_=ldv, func=AF.Exp, scale=-0.5)
            xnh = xnp.tile([128, C], bf16, name="xnh", tag="xnh")
            nc.vector.tensor_scalar(out=xnh, in0=xt, scalar1=mv_t[:, 0:1],
                                    scalar2=rstd, op0=OP.subtract, op1=OP.mult)
            return xnh

        # ================= Phase A: TimeMix =================
        with ExitStack() as ph:
            xtp = ph.enter_context(tc.tile_pool(name="xtp", bufs=12))
            carp = ph.enter_context(tc.tile_pool(name="carp", bufs=1))
            statsp = ph.enter_context(tc.tile_pool(name="statsA", bufs=4))
            xnp = ph.enter_context(tc.tile_pool(name="xnpA", bufs=2))
            xnTp = ph.enter_context(tc.tile_pool(name="xnTA", bufs=2))
            mixp = ph.enter_context(tc.tile_pool(name="mixA", bufs=2))
            wkvp = ph.enter_context(tc.tile_pool(name="wkv", bufs=2))
            x2p = ph.enter_context(tc.tile_pool(name="x2p", bufs=2))
            wp = ph.enter_context(tc.tile_pool(name="wA", bufs=1))
            psA = ph.enter_context(tc.tile_pool(name="psA", bufs=5, space="PSUM"))
            psO = ph.enter_context(tc.tile_pool(name="psO", bufs=2, space="PSUM"))

            NTT = TCA // 128  # 4 token tiles per chunk

            # First chunk's x tiles + params first so compute starts ASAP,
            # then weights in use-order (wk -> wv -> wr -> wo), one big DMA
            # each.
            PD = 16

            def load_x(ic):
                # alternate between the two hwdge queues so tile loads are
                # not serialized behind each other or the XBARs
                xts = []
                for tt in range(NTT):
                    xt = xtp.tile([128, C], bf16, name="xt", tag="xt")
                    eng = nc.sync if tt % 2 == 0 else nc.scalar
                    eng.dma_start(
                        out=xt,
                        in_=x_d[ic * TCA + tt * 128: ic * TCA + (tt + 1) * 128, :])
                    xts.append(xt)
                return xts

            def ln_xbar(ic, xts, prev_xnT):
                # feature-major, front-padded for token_shift. The XBAR
                # transpose needs a 32B-aligned destination, so data starts
                # at column PD=16 and the shift-pad lives at column 15.
                xnT = xnTp.tile([128, CB, TCA + PD], bf16, name="xnT", tag="xnT")
                if ic == 0:
                    nc.vector.memset(xnT[:, :, PD - 1], 0.0)
                else:
                    nc.scalar.copy(out=xnT[:, :, PD - 1],
                                   in_=prev_xnT[:, :, PD - 1 + TCA])
                for tt in range(NTT):
                    xnh = ln_apply(xts[tt], statsp, xnp)
                    # XBAR transpose [128 tok, C] -> [128 chan, cb, 128 tok]
                    nc.sync.dma_start(
                        out=xnT[:, :, PD + tt * 128: PD + (tt + 1) * 128],
                        in_=xnh, transpose=True)
                return xnT

            xts = load_x(0)
            xts_nxt = load_x(1)
            nc.sync.dma_start(out=pars_t, in_=pars_d)
            nc.vector.memset(eps_t, LN_EPS)
            nc.vector.memset(one_t, 1.0)

            # Bulk weight DMAs go through the ACT hwdge queue so the
            # latency-critical XBAR transposes on the SP queue are not stuck
            # behind 8MB of weight traffic.
            wk_sb = wp.tile([128, CB, C], bf16)
            wv_sb = wp.tile([128, CB, C], bf16)
            wr_sb = wp.tile([128, CB, C], bf16)
            wo_sb = wp.tile([128, CB, C], bf16)
            for w_sb, w_d in ((wk_sb, wk_d), (wv_sb, wv_d),
                              (wr_sb, wr_d), (wo_sb, wo_d)):
                nc.gpsimd.dma_start(out=w_sb, in_=w_d)

            # Cross-chunk scan carries live in a persistent [128, 2*CB] tile
            # (ABpad buffers rotate within a chunk, so a prev-chunk tile
            # reference would be recycled before the next chunk reads it).
            # Layout: [:, 2*db] = A carry, [:, 2*db+1] = B carry.
            ABcar = carp.tile([128, 2 * CB], f32)

            def mix_chunk(xnT):
                # xk on DVE (single stt); xv/xr on the otherwise-idle GpSimd
                # as mul+add pairs — the DVE is phase A's long pole and the
                # mixes are pipelined a full chunk ahead so GpSimd's latency
                # is hidden under the previous chunk's matmuls.
                xk_t = mixp.tile([128, CB, TCA], bf16, name="xk_t", tag="xk")
                xv_t = mixp.tile([128, CB, TCA], bf16, name="xv_t", tag="xv")
                xr_t = mixp.tile([128, CB, TCA], bf16, name="xr_t", tag="xr")
                for cb in range(CB):
                    xn_v = xnT[:, cb, PD:PD + TCA]          # xn[t]
                    xx_v = xnT[:, cb, PD - 1:PD - 1 + TCA]  # xn[t-1]
                    dd = mixp.tile([128, TCA], bf16, name="dd", tag="dd", bufs=2)
                    nc.vector.tensor_sub(dd, xn_v, xx_v)
                    nc.vector.scalar_tensor_tensor(
                        out=xk_t[:, cb, :], in0=dd, scalar=par["mk"][:, cb:cb + 1],
                        in1=xx_v, op0=OP.mult, op1=OP.add)
                    nc.gpsimd.tensor_scalar_mul(
                        xv_t[:, cb, :], dd, par["mv"][:, cb:cb + 1])
                    nc.gpsimd.tensor_add(xv_t[:, cb, :], xv_t[:, cb, :], xx_v)
                    nc.gpsimd.tensor_scalar_mul(
                        xr_t[:, cb, :], dd, par["mr"][:, cb:cb + 1])
                    nc.gpsimd.tensor_add(xr_t[:, cb, :], xr_t[:, cb, :], xx_v)
                return xk_t, xv_t, xr_t

            xnT = ln_xbar(0, xts, None)
            mix = mix_chunk(xnT)
            xnT_nxt = None
            mix_nxt = None

            for ic in range(NCHA):
                _sid, _ = nc.enter_named_scope(f"A{ic}", False)
                t0 = ic * TCA
                xk_t, xv_t, xr_t = mix

                # Pipelined LN+XBAR+mixes for the NEXT chunk, emitted before
                # this chunk's scan tail so the DVE/ACT/Pool FIFOs don't
                # serialize the next chunk's inputs behind it.
                if ic + 1 < NCHA:
                    xnT_nxt = ln_xbar(ic + 1, xts_nxt, xnT)
                    mix_nxt = mix_chunk(xnT_nxt)
                    xts_prev = xts
                    xts = xts_nxt
                    if ic + 2 < NCHA:
                        xts_nxt = load_x(ic + 2)
                else:
                    xts_prev = xts

                rw_t = wkvp.tile([128, CB, TCA], bf16, name="rw_t", tag="rw")
                for db in range(CB):
                    dsl = slice(db * 128, (db + 1) * 128)
                    psk = psA.tile([128, TCA], f32, name="psk", tag="mmA")
                    for cb in range(CB):
                        nc.tensor.matmul(psk, wk_sb[:, cb, dsl], xk_t[:, cb, :],
                                         start=(cb == 0), stop=(cb == CB - 1))
                    # Two exp flavors: ek = e^k feeds the scans; eku = e^{k+u}
                    # is the instantaneous term in num/den (u = time_first,
                    # folded into the bias host-side: bku = bk + u).
                    ek = wkvp.tile([128, TCA], bf16, name="ek", tag="ek")
                    nc.scalar.activation(out=ek, in_=psk, func=AF.Exp,
                                         bias=par["bk"][:, db:db + 1])
                    eku = wkvp.tile([128, TCA], bf16, name="eku", tag="eku")
                    nc.scalar.activation(out=eku, in_=psk, func=AF.Exp,
                                         bias=par["bku"][:, db:db + 1])

                    psv = psA.tile([128, TCA], f32, name="psv", tag="mmA")
                    for cb in range(CB):
                        nc.tensor.matmul(psv, wv_sb[:, cb, dsl], xv_t[:, cb, :],
                                         start=(cb == 0), stop=(cb == CB - 1))
                    vlin = wkvp.tile([128, TCA], bf16, name="vlin", tag="vlin")
                    nc.scalar.activation(out=vlin, in_=psv, func=AF.Identity,
                                         bias=par["bv"][:, db:db + 1])
                    ekv = wkvp.tile([128, TCA], bf16, name="ekv", tag="ekv")
                    nc.vector.tensor_mul(ekv, vlin, ek)
                    ekvu = wkvp.tile([128, TCA], bf16, name="ekvu", tag="ekvu")
                    nc.vector.tensor_mul(ekvu, vlin, eku)

                    psr = psA.tile([128, TCA], f32, name="psr", tag="mmA")
                    for cb in range(CB):
                        nc.tensor.matmul(psr, wr_sb[:, cb, dsl], xr_t[:, cb, :],
                                         start=(cb == 0), stop=(cb == CB - 1))
                    # sigmoid(r)*num/den == num / (den*(1+exp(-r))) — avoids
                    # the Sigmoid table set; everything stays in ln/exp.
                    er = wkvp.tile([128, TCA], bf16, name="er", tag="sr")
                    nc.scalar.activation(out=er, in_=psr, func=AF.Exp, scale=-1.0,
                                         bias=par["brn"][:, db:db + 1])

                    lam_s = par["lam"][:, db:db + 1]
                    # A and B scans share one padded tile so the carry
                    # copy-in/copy-out is a single [128, 2] ACT op each.
                    ABpad = wkvp.tile([128, 2, TCA + 1], bf16, name="ABpad",
                                      tag="ABpad")
                    if ic == 0:
                        nc.vector.memset(ABpad[:, :, 0], 0.0)
                        a_init = 0.0
                        b_init = 0.0
                    else:
                        nc.scalar.copy(out=ABpad[:, :, 0],
                                       in_=ABcar[:, 2 * db:2 * db + 2])
                        a_init = ABcar[:, 2 * db:2 * db + 1]
                        b_init = ABcar[:, 2 * db + 1:2 * db + 2]
                    nc.vector.tensor_tensor_scan(
                        out=ABpad[:, 0, 1:TCA + 1],
                        data0=lam_s.broadcast_to([128, TCA]),
                        data1=ekv, initial=a_init, op0=OP.mult, op1=OP.add)
                    nc.vector.tensor_tensor_scan(
                        out=ABpad[:, 1, 1:TCA + 1],
                        data0=lam_s.broadcast_to([128, TCA]),
                        data1=ek, initial=b_init, op0=OP.mult, op1=OP.add)
                    if ic < NCHA - 1:
                        nc.scalar.copy(out=ABcar[:, 2 * db:2 * db + 2],
                                       in_=ABpad[:, :, TCA])

                    # Tail (all-bf16 tensor_tensor ops hit the DVE 2x mode):
                    #   num -> ekvu (+= A_prev), den -> eku (+= B_prev)
                    #   1/(den*(1+er)) = exp(-(ln(1+er) + ln(den)))
                    nc.vector.tensor_add(ekvu, ekvu, ABpad[:, 0, 0:TCA])
                    nc.vector.tensor_add(eku, eku, ABpad[:, 1, 0:TCA])
                    l1 = wkvp.tile([128, TCA], f32, name="l1", tag="l1")
                    nc.scalar.activation(out=l1, in_=er, func=AF.Ln, bias=one_t)
                    lden = wkvp.tile([128, TCA], f32, name="lden", tag="lden")
                    nc.scalar.activation(out=lden, in_=eku, func=AF.Ln)
                    nc.vector.tensor_add(l1, l1, lden)
                    rinv = wkvp.tile([128, TCA], bf16, name="rinv", tag="rinv")
                    nc.scalar.activation(out=rinv, in_=l1, func=AF.Exp,
                                         scale=-1.0)
                    nc.vector.tensor_mul(rw_t[:, db, :], ekvu, rinv)

                # Wo back to token-major + residual
                for tt in range(NTT):
                    for chh in range(2):
                        pso = psO.tile([128, 512], f32, name="pso", tag="mmO")
                        for db in range(CB):
                            nc.tensor.matmul(
                                pso,
                                rw_t[:, db, tt * 128:(tt + 1) * 128],
                                wo_sb[:, db, chh * 512:(chh + 1) * 512],
                                start=(db == 0), stop=(db == CB - 1))
                        x2sb = x2p.tile([128, 512], bf16, name="x2sb", tag="x2")
                        nc.vector.tensor_add(
                            x2sb, xts_prev[tt][:, chh * 512:(chh + 1) * 512], pso)
                        jb = ic * 2 + tt // 2
                        row = (tt % 2) * 128
                        nc.sync.dma_start(
                            out=x2_dram[jb][row:row + 128,
                                            chh * 512:(chh + 1) * 512],
                            in_=x2sb)
                xnT = xnT_nxt
                mix = mix_nxt
                nc.leave_named_scope(f"A{ic}", _sid, False)

        # ================= Phase B: ChannelMix =================
        with ExitStack() as ph:
            x2tp = ph.enter_context(tc.tile_pool(name="x2tp", bufs=5))
            statsp = ph.enter_context(tc.tile_pool(name="statsB", bufs=4))
            xnp = ph.enter_context(tc.tile_pool(name="xnpB", bufs=2))
            xnTp = ph.enter_context(tc.tile_pool(name="xnTB", bufs=2))
            mixp = ph.enter_context(tc.tile_pool(name="mixB", bufs=1))
            ffp = ph.enter_context(tc.tile_pool(name="ffp", bufs=2))
            wp = ph.enter_context(tc.tile_pool(name="wB", bufs=1))
            psKV = ph.enter_context(tc.tile_pool(name="psKV", bufs=1, space="PSUM"))
            psKF = ph.enter_context(tc.tile_pool(name="psKF", bufs=1, space="PSUM"))
            psSm = ph.enter_context(tc.tile_pool(name="psSm", bufs=2, space="PSUM"))

            NTT = TCB // 128  # 2
            PD = 16

            def load_x2(jc):
                x2ts = []
                for tt in range(NTT):
                    x2t = x2tp.tile([128, C], bf16, name="x2t", tag="x2t")
                    eng = nc.sync if tt % 2 == 0 else nc.scalar
                    eng.dma_start(
                        out=x2t,
                        in_=x2_dram[jc][tt * 128:(tt + 1) * 128, :])
                    x2ts.append(x2t)
                return x2ts

            def ln_xbar2(jc, x2ts, prev_xnT):
                xnT = xnTp.tile([128, CB, TCB + PD], bf16, name="xnT2",
                                tag="xnT2")
                if jc == 0:
                    nc.vector.memset(xnT[:, :, PD - 1], 0.0)
                else:
                    nc.scalar.copy(out=xnT[:, :, PD - 1],
                                   in_=prev_xnT[:, :, PD - 1 + TCB])
                for tt in range(NTT):
                    xnh = ln_apply(x2ts[tt], statsp, xnp)
                    nc.sync.dma_start(
                        out=xnT[:, :, PD + tt * 128: PD + (tt + 1) * 128],
                        in_=xnh, transpose=True)
                return xnT

            x2ts = load_x2(0)
            x2ts_nxt = load_x2(1)
            if with_b2r:
                b2r_bcast = bass.AP(tensor=b2r_d.tensor, offset=b2r_d.offset,
                                    ap=[[0, 128]] + list(b2r_d.ap))
                nc.sync.dma_start(out=b2r_t, in_=b2r_bcast)

            # Weights streamed in use-order on the ACT hwdge queue: wkf
            # (f-quarter-major host layout so each quarter is one descriptor
            # per partition AND kf can start after the first lands), wrf,
            # then wvf under the kf MMs.
            wkf_sb = wp.tile([128, 4, CB, FFN // 4], bf16)
            wrf_sb = wp.tile([128, CB, C], bf16)
            wvf_sb = wp.tile([128, FB, C], bf16)
            for q in range(4):
                nc.gpsimd.dma_start(out=wkf_sb[:, q], in_=wkf_d[:, q])
                if q == 0:
                    nc.gpsimd.dma_start(out=wrf_sb, in_=wrf_d)
            for q in range(4):
                nc.gpsimd.dma_start(
                    out=wvf_sb[:, q * (FB // 4):(q + 1) * (FB // 4), :],
                    in_=wvf_d[:, q * (FB // 4):(q + 1) * (FB // 4), :])

            def mix2_chunk(xnT):
                xk_t = mixp.tile([128, CB, TCB], bf16, name="xk2_t", tag="xk2")
                xr_t = mixp.tile([128, CB, TCB], bf16, name="xr2_t", tag="xr2")
                for cb in range(CB):
                    xn_v = xnT[:, cb, PD:PD + TCB]
                    xx_v = xnT[:, cb, PD - 1:PD - 1 + TCB]
                    dd = mixp.tile([128, TCB], bf16, name="dd2", tag="dd2", bufs=1)
                    nc.vector.tensor_sub(dd, xn_v, xx_v)
                    nc.vector.scalar_tensor_tensor(
                        out=xk_t[:, cb, :], in0=dd, scalar=par["mfk"][:, cb:cb + 1],
                        in1=xx_v, op0=OP.mult, op1=OP.add)
                    nc.vector.scalar_tensor_tensor(
                        out=xr_t[:, cb, :], in0=dd, scalar=par["mfr"][:, cb:cb + 1],
                        in1=xx_v, op0=OP.mult, op1=OP.add)
                return xk_t, xr_t

            xnT = ln_xbar2(0, x2ts, None)
            mix = mix2_chunk(xnT)
            xnT_nxt = None
            mix_nxt = None

            for jc in range(NCHB):
                _sid, _ = nc.enter_named_scope(f"B{jc}", False)
                t0 = jc * TCB
                xk_t, xr_t = mix

                # Pipelined LN+XBAR+mixes for the next chunk (same reasoning
                # as phase A: keep the DVE/ACT FIFOs from serializing the
                # next chunk's inputs behind this chunk's elementwise tail).
                if jc + 1 < NCHB:
                    xnT_nxt = ln_xbar2(jc + 1, x2ts_nxt, xnT)
                    mix_nxt = mix2_chunk(xnT_nxt)
                    x2ts_prev = x2ts
                    x2ts = x2ts_nxt
                    if jc + 2 < NCHB:
                        x2ts_nxt = load_x2(jc + 2)
                else:
                    x2ts_prev = x2ts

                # kf for all 32 f-blocks first (one long PE run), then kv as
                # sequential psum groups. kfs is held in two half tiles with
                # bufs=3 (1.5-buffering): the next chunk's kf half-0 can start
                # as soon as this chunk's kv has consumed fb 0..15.
                FBH = FB // 2
                kfs_h = [ffp.tile([128, FBH, TCB], bf16, name=f"kfs{h}",
                                  tag="kfs", bufs=3) for h in range(2)]
                for fb in range(FB):
                    pskf = psKF.tile([128, TCB], f32, name="pskf", tag="kfps",
                                     bufs=4)
                    for cb in range(CB):
                        nc.tensor.matmul(
                            pskf,
                            wkf_sb[:, fb // 8, cb,
                                   (fb % 8) * 128:(fb % 8 + 1) * 128],
                            xk_t[:, cb, :], start=(cb == 0), stop=(cb == CB - 1))
                    kfb = ffp.tile([128, TCB], bf16, name="kfb", tag="kf", bufs=2)
                    nc.scalar.activation(out=kfb, in_=pskf, func=AF.Relu,
                                         bias=bkf_t[:, fb:fb + 1])
                    nc.vector.tensor_mul(kfs_h[fb // FBH][:, fb % FBH, :],
                                         kfb, kfb)

                for tt in range(NTT):
                    for chh in range(2):
                        kvp = psKV.tile([128, 512], f32, name="kvp", tag="kv",
                                        bufs=2)
                        for fb in range(FB):
                            nc.tensor.matmul(
                                kvp,
                                kfs_h[fb // FBH][:, fb % FBH,
                                                 tt * 128:(tt + 1) * 128],
                                wvf_sb[:, fb, chh * 512:(chh + 1) * 512],
                                start=(fb == 0), stop=(fb == FB - 1))
                        psr2 = psSm.tile([128, 512], f32, name="psr2", tag="sm")
                        for cb in range(CB):
                            nc.tensor.matmul(
                                psr2,
                                xr_t[:, cb, tt * 128:(tt + 1) * 128],
                                wrf_sb[:, cb, chh * 512:(chh + 1) * 512],
                                start=(cb == 0), stop=(cb == CB - 1))
                        # sigmoid via exp/ln only (same ACT table set):
                        # sig = exp(-ln(1+exp(-r)))
                        if with_b2r:
                            nc.vector.tensor_add(
                                psr2, psr2,
                                b2r_t[:, chh * 512:(chh + 1) * 512])
                        er2 = ffp.tile([128, 512], bf16, name="er2", tag="sr2")
                        nc.scalar.activation(out=er2, in_=psr2, func=AF.Exp,
                                             scale=-1.0)
                        nc.scalar.activation(out=er2, in_=er2, func=AF.Ln,
                                             bias=one_t)
                        nc.scalar.activation(out=er2, in_=er2, func=AF.Exp,
                                             scale=-1.0)
                        tmp = ffp.tile([128, 512], f32, name="tmp", tag="tmp", bufs=2)
                        nc.vector.tensor_mul(tmp, er2, kvp)
                        nc.vector.tensor_add(
                            tmp, tmp,
                            x2ts_prev[tt][:, chh * 512:(chh + 1) * 512])
                        nc.sync.dma_start(
                            out=y_d[t0 + tt * 128: t0 + (tt + 1) * 128,
                                    chh * 512:(chh + 1) * 512],
                            in_=tmp)
                xnT = xnT_nxt
                mix = mix_nxt
                nc.leave_named_scope(f"B{jc}", _sid, False)

    nc.compile()
    return nc


def get_program(with_b2r=False):
    key = ("nc", with_b2r)
    if key not in _CACHE:
        _CACHE[key] = _build_program(with_b2r)
    return _CACHE[key]


def host_inputs(inputs):
    """Host-side precompute: transposed bf16 weights + derived per-channel
    vectors. Returns the shared (per-core-identical) part of the in_map."""
    bf = ml_dtypes.bfloat16
    f32 = np.float32

    def v(name):
        return np.asarray(inputs[name], f32).reshape(-1)

    g1, b1 = v("ln1_g"), v("ln1_b")
    g2, b2 = v("ln2_g"), v("ln2_b")
    Wk = np.asarray(inputs["Wk"], f32)
    Wv = np.asarray(inputs["Wv"], f32)
    Wr = np.asarray(inputs["Wr"], f32)
    Wkf = np.asarray(inputs["Wk_ffn"], f32)
    Wrf = np.asarray(inputs["Wr_ffn"], f32)

    # The XBAR transpose writes standard channel blocking: transposed row
    # c = cb*128 + p lands at [partition p, block cb] — same layout as the
    # weight DMAs, so no permutation is needed anywhere.
    def tb(a):
        # [in_ch, cols] -> SBUF tile layout [128, in_blocks, cols]
        # (partition-contiguous so the DMA is 1 descriptor per partition)
        n, m = a.shape
        return np.ascontiguousarray(
            a.reshape(n // 128, 128, m).transpose(1, 0, 2).astype(bf))

    def col(x):  # channel panel [128, n]: entry [p, a] = x[a*128 + p]
        return np.ascontiguousarray(x.reshape(-1, 128).T)

    pars = np.concatenate([
        col(v("att_mix_k")), col(v("att_mix_v")), col(v("att_mix_r")),
        col(v("ffn_mix_k")), col(v("ffn_mix_r")),
        col(np.exp(-np.exp(v("time_decay"))).astype(f32)),
        col((Wk @ b1).astype(f32)),
        # u = time_first folded into the bias of the second exp flavor:
        # eku = exp(psk + Wk@b1 + u)
        col((Wk @ b1 + v("time_first")).astype(f32)),
        col((Wv @ b1).astype(f32)),
        col((-(Wr @ b1)).astype(f32)),
        col((Wkf @ b2).astype(f32)),
    ], axis=1).astype(f32)
    assert pars.shape == (128, NPARC)

    shared = {
        # LN gain folded in: row c of W.T scaled by g[c]
        "wkT": tb(Wk.T * g1[:, None]),
        "wvT": tb(Wv.T * g1[:, None]),
        "wrT": tb(Wr.T * g1[:, None]),
        "woT": tb(np.asarray(inputs["Wo"], f32).T),
        # wkf additionally split f-quarter-major: [128, 4, CB, FFN//4]
        "wkfT": np.ascontiguousarray(
            tb(Wkf.T * g2[:, None]).reshape(128, CB, 4, FFN // 4)
            .transpose(0, 2, 1, 3)),
        "wvfT": tb(np.asarray(inputs["Wv_ffn"], f32).T),
        "wrfT": tb(Wrf.T * g2[:, None]),
        "pars": np.ascontiguousarray(pars),
    }
    b2r = (Wrf @ b2).astype(f32)
    with_b2r = bool(np.any(b2r != 0.0))
    if with_b2r:
        shared["b2r"] = b2r
    return shared, with_b2r


def _ensure_axon_hooks():
    """The agent image's antenv lacks axon_hooks; bass_utils trace=True needs
    it. Install a shim wired to the injected libaxon_pjrt.so NTFF profiler."""
    try:
        import antenv.axon_hooks  # noqa: F401
        return
    except ImportError:
        pass
    import types
    mod = types.ModuleType("antenv.axon_hooks")
    mod._hook = None

    def set_axon_ntff_profile_hook(h):
        mod._hook = h

    def get_axon_ntff_profile_hook():
        return mod._hook

    mod.set_axon_ntff_profile_hook = set_axon_ntff_profile_hook
    mod.get_axon_ntff_profile_hook = get_axon_ntff_profile_hook
    sys.modules["antenv.axon_hooks"] = mod
    import antenv
    antenv.axon_hooks = mod
    try:
        from trn_agent_boot.trn_boot import _ntff_profile_via_ctypes
        so = "/opt/axon/libaxon_pjrt.so"
        if os.path.exists(so):
            mod._hook = _ntff_profile_via_ctypes(so)
    except Exception as e:  # pragma: no cover - degrade to no tracing
        print("ntff hook unavailable:", e)


def kernel(**inputs):
    from concourse import bass_utils

    shared, with_b2r = host_inputs(inputs)
    nc = get_program(with_b2r)
    X = np.asarray(inputs["x"], np.float32).astype(ml_dtypes.bfloat16)
    in_maps = [dict(shared, x=np.ascontiguousarray(X[b])) for b in range(NCORES)]

    trace = os.environ.get("KERNEL_TRACE", "0") == "1"
    if trace:
        _ensure_axon_hooks()
    res = bass_utils.run_bass_kernel_spmd(
        nc, in_maps, core_ids=list(range(NCORES)), trace=trace)
    kernel._last_exec_ns = res.exec_time_ns
    kernel._last_results = res
    out = np.stack([r["y"] for r in res.results], axis=0)
    return out


kernel._last_exec_ns = None


# revision 58
# speedup vs baseline: 1.5046x; 1.5046x over previous
"""RWKV-4 block (TimeMix + ChannelMix) Trainium2 Bass kernel.

Full inputs in, full outputs out. Sharding: data-parallel over batch B=8
across the 8 NeuronCores (one batch element per core, no collectives).

Per-core layout strategy:
  - LayerNorm + stats in token-major [128 tokens, C] (bn_stats along free
    dim), then XBAR DMA-transpose to feature-major [128 chan, T] in one op
    per token tile. Channels are interleaved across partitions
    (channel c = p*8 + cb lives at [partition p, block cb]) so the XBAR's
    row-write order matches the [p, cb, t] tile layout; weight rows and
    per-channel mix params are permuted host-side to match.
  - token_shift is a free-dim offset view of a front-padded feature-major
    tile; the pad column is copied from the previous chunk's last column.
  - k/v/r and FFN matmuls in bf16 (fp32 PSUM accumulation).
  - WKV recurrence A_t = lam*A_{t-1} + e^{k_t+u} v_t (and B with e^{k_t+u})
    runs as a native DVE tensor_tensor_scan per 128-channel block, fp32,
    chained across T-chunks via the `initial` operand (prev scan's last
    element). e^u is folded into the exp bias host-side: it cancels in
    y = num/den, so num = ekv + A_prev and den = ek + B_prev are plain adds.
    Division via ACT Ln+Exp (1/x = exp(-ln x), denominator positive).
  - Elementwise work is spread across engines: DVE (scans, adds, k-mix),
    GpSimd (v/r mixes, subs, rw mult), ACT (exp/ln, psv+bv via Copy bias).
"""

import os
import sys

import numpy as np

if "/opt/trn_rl_repo" not in sys.path:
    sys.path.insert(0, "/opt/trn_rl_repo")

import ml_dtypes

B, T, C, FFN = 8, 2048, 1024, 4096
NCORES = 8
CB = C // 128      # 8 channel blocks
FB = FFN // 128    # 32 ffn blocks
TCA = 512          # TimeMix chunk (tokens)
NCHA = T // TCA    # 4
TCB = 256          # FFN chunk (tokens)
NCHB = T // TCB    # 8
LN_EPS = 1e-5

_CACHE = {}

# pars packing: 10 per-C params as [128, 8] panels + bkf as [128, 32]
PAR_NAMES = ["mk", "mv", "mr", "mfk", "mfr", "lam", "bk", "bku", "bv", "brn"]
NPARC = len(PAR_NAMES) * CB + FB  # 112


def _build_program(with_b2r=False):
    import concourse.bass as bass  # noqa: F401
    from concourse import bacc
    import concourse.tile as tile
    import concourse.mybir as mybir
    from contextlib import ExitStack

    f32 = mybir.dt.float32
    bf16 = mybir.dt.bfloat16
    AF = mybir.ActivationFunctionType
    OP = mybir.AluOpType

    # Pin ALL activations to the one table set that contains every function
    # we use (ln/exp/relu/copy). Without this the table-load pass picks the
    # first set containing each function and thrashes ACT_TABLE_LOADs
    # (~2.7us each).
    import concourse.hw_specs as hw_specs
    if not getattr(hw_specs, "_rwkv_patched", False):
        _orig_gat = hw_specs.get_activation_tables

        def _only_lnexp(arch):
            t = _orig_gat(arch)
            keep = "natural_log_exp_and_others"
            return {name: (funcs if name == keep else set())
                    for name, funcs in t.items()}

        hw_specs.get_activation_tables = _only_lnexp
        bacc.get_activation_tables = _only_lnexp
        hw_specs._rwkv_patched = True

    nc = bacc.Bacc("TRN2", target_bir_lowering=False, debug=False,
                   enable_asserts=False)

    # ---------------- DRAM I/O ----------------
    x_d = nc.dram_tensor("x", [T, C], bf16, kind="ExternalInput").ap()
    y_d = nc.dram_tensor("y", [T, C], f32, kind="ExternalOutput").ap()

    # Weights are pre-arranged host-side to the SBUF tile layout
    # [128, blocks, cols] (partition-contiguous) so each weight DMA needs
    # only one descriptor per partition — descriptor generation for the
    # naive strided layout serialized the SP queue for ~6us per transfer.
    wk_d = nc.dram_tensor("wkT", [128, CB, C], bf16, kind="ExternalInput").ap()
    wv_d = nc.dram_tensor("wvT", [128, CB, C], bf16, kind="ExternalInput").ap()
    wr_d = nc.dram_tensor("wrT", [128, CB, C], bf16, kind="ExternalInput").ap()
    wo_d = nc.dram_tensor("woT", [128, CB, C], bf16, kind="ExternalInput").ap()
    wkf_d = nc.dram_tensor("wkfT", [128, 4, CB, FFN // 4], bf16,
                           kind="ExternalInput").ap()
    wvf_d = nc.dram_tensor("wvfT", [128, FB, C], bf16, kind="ExternalInput").ap()
    wrf_d = nc.dram_tensor("wrfT", [128, CB, C], bf16, kind="ExternalInput").ap()

    # All small per-channel params packed into ONE [128, NPARC] fp32 tensor
    # (a single contiguous DMA instead of ~11 tiny strided ones).
    pars_d = nc.dram_tensor("pars", [128, NPARC], f32, kind="ExternalInput").ap()
    if with_b2r:
        b2r_d = nc.dram_tensor("b2r", [C], f32, kind="ExternalInput").ap()

    with tile.TileContext(nc) as tc, ExitStack() as glob:
        const = glob.enter_context(tc.tile_pool(name="const", bufs=1))
        dram = glob.enter_context(tc.tile_pool(name="dram", bufs=1, space="DRAM"))

        pars_t = const.tile([128, NPARC], f32)
        par = {}
        for i, p in enumerate(PAR_NAMES):
            par[p] = pars_t[:, i * CB:(i + 1) * CB]
        bkf_t = pars_t[:, len(PAR_NAMES) * CB:len(PAR_NAMES) * CB + FB]
        if with_b2r:
            b2r_t = const.tile([128, C], f32)

        eps_t = const.tile([128, 1], f32)
        one_t = const.tile([128, 1], f32)

        # x2 is split per B-chunk so phase B's reads only depend on the
        # A-chunk that produced those tokens (whole-tile DRAM deps would
        # stall B0 until all of A finished).
        x2_dram = [dram.tile([TCB, C], bf16, name=f"x2d{j}")
                   for j in range(NCHB)]

        # ---- helper: LN stats + apply for one [128, C] token tile ----
        def ln_apply(xt, statsp, xnp):
            st6 = statsp.tile([128, 2, 6], f32, name="st6", tag="st6")
            nc.vector.bn_stats(out=st6[:, 0, :], in_=xt[:, 0:512])
            nc.vector.bn_stats(out=st6[:, 1, :], in_=xt[:, 512:1024])
            mv_t = statsp.tile([128, 2], f32, name="mv_t", tag="mv")
            nc.vector.bn_aggr(out=mv_t, in_=st6)
            # rstd = exp(-0.5*ln(var+eps)) — keeps ACT inside the ln/exp
            # table set (Sqrt lives in a different set -> ~2.7us reload).
            ldv = statsp.tile([128, 1], f32, name="ldv", tag="ldv")
            nc.scalar.activation(out=ldv, in_=mv_t[:, 1:2], func=AF.Ln,
                                 bias=eps_t)
            rstd = statsp.tile([128, 1], f32, name="rstd", tag="rstd")
            nc.scalar.activation(out=rstd, in_=ldv, func=AF.Exp, scale=-0.5)
            xnh = xnp.tile([128, C], bf16, name="xnh", tag="xnh")
            nc.vector.tensor_scalar(out=xnh, in0=xt, scalar1=mv_t[:, 0:1],
                                    scalar2=rstd, op0=OP.subtract, op1=OP.mult)
            return xnh

        # ================= Phase A: TimeMix =================
        with ExitStack() as ph:
            xtp = ph.enter_context(tc.tile_pool(name="xtp", bufs=12))
            carp = ph.enter_context(tc.tile_pool(name="carp", bufs=1))
            statsp = ph.enter_context(tc.tile_pool(name="statsA", bufs=4))
            xnp = ph.enter_context(tc.tile_pool(name="xnpA", bufs=2))
            xnTp = ph.enter_context(tc.tile_pool(name="xnTA", bufs=2))
            mixp = ph.enter_context(tc.tile_pool(name="mixA", bufs=2))
            wkvp = ph.enter_context(tc.tile_pool(name="wkv", bufs=2))
            x2p = ph.enter_context(tc.tile_pool(name="x2p", bufs=2))
            wp = ph.enter_context(tc.tile_pool(name="wA", bufs=1))
            psA = ph.enter_context(tc.tile_pool(name="psA", bufs=5, space="PSUM"))
            psO = ph.enter_context(tc.tile_pool(name="psO", bufs=2, space="PSUM"))

            NTT = TCA // 128  # 4 token tiles per chunk

            # First chunk's x tiles + params first so compute starts ASAP,
            # then weights in use-order (wk -> wv -> wr -> wo), one big DMA
            # each.
            PD = 16

            def load_x(ic):
                # alternate between the two hwdge queues so tile loads are
                # not serialized behind each other or the XBARs
                xts = []
                for tt in range(NTT):
                    xt = xtp.tile([128, C], bf16, name="xt", tag="xt")
                    eng = nc.sync if tt % 2 == 0 else nc.scalar
                    eng.dma_start(
                        out=xt,
                        in_=x_d[ic * TCA + tt * 128: ic * TCA + (tt + 1) * 128, :])
                    xts.append(xt)
                return xts

            def ln_xbar(ic, xts, prev_xnT):
                # feature-major, front-padded for token_shift. The XBAR
                # transpose needs a 32B-aligned destination, so data starts
                # at column PD=16 and the shift-pad lives at column 15.
                xnT = xnTp.tile([128, CB, TCA + PD], bf16, name="xnT", tag="xnT")
                if ic == 0:
                    nc.vector.memset(xnT[:, :, PD - 1], 0.0)
                else:
                    nc.scalar.copy(out=xnT[:, :, PD - 1],
                                   in_=prev_xnT[:, :, PD - 1 + TCA])
                for tt in range(NTT):
                    xnh = ln_apply(xts[tt], statsp, xnp)
                    # XBAR transpose [128 tok, C] -> [128 chan, cb, 128 tok]
                    nc.sync.dma_start(
                        out=xnT[:, :, PD + tt * 128: PD + (tt + 1) * 128],
                        in_=xnh, transpose=True)
                return xnT

            xts = load_x(0)
            xts_nxt = load_x(1)
            nc.sync.dma_start(out=pars_t, in_=pars_d)
            nc.vector.memset(eps_t, LN_EPS)
            nc.vector.memset(one_t, 1.0)

            # Bulk weight DMAs go through the ACT hwdge queue so the
            # latency-critical XBAR transposes on the SP queue are not stuck
            # behind 8MB of weight traffic.
            wk_sb = wp.tile([128, CB, C], bf16)
            wv_sb = wp.tile([128, CB, C], bf16)
            wr_sb = wp.tile([128, CB, C], bf16)
            wo_sb = wp.tile([128, CB, C], bf16)
            for w_sb, w_d in ((wk_sb, wk_d), (wv_sb, wv_d),
                              (wr_sb, wr_d), (wo_sb, wo_d)):
                nc.gpsimd.dma_start(out=w_sb, in_=w_d)

            # Cross-chunk scan carries live in a persistent [128, 2*CB] tile
            # (ABpad buffers rotate within a chunk, so a prev-chunk tile
            # reference would be recycled before the next chunk reads it).
            # Layout: [:, 2*db] = A carry, [:, 2*db+1] = B carry.
            ABcar = carp.tile([128, 2 * CB], f32)

            def mix_chunk(xnT):
                # xk on DVE (single stt); xv/xr on the otherwise-idle GpSimd
                # as mul+add pairs — the DVE is phase A's long pole and the
                # mixes are pipelined a full chunk ahead so GpSimd's latency
                # is hidden under the previous chunk's matmuls.
                xk_t = mixp.tile([128, CB, TCA], bf16, name="xk_t", tag="xk")
                xv_t = mixp.tile([128, CB, TCA], bf16, name="xv_t", tag="xv")
                xr_t = mixp.tile([128, CB, TCA], bf16, name="xr_t", tag="xr")
                for cb in range(CB):
                    xn_v = xnT[:, cb, PD:PD + TCA]          # xn[t]
                    xx_v = xnT[:, cb, PD - 1:PD - 1 + TCA]  # xn[t-1]
                    dd = mixp.tile([128, TCA], bf16, name="dd", tag="dd", bufs=2)
                    nc.vector.tensor_sub(dd, xn_v, xx_v)
                    nc.vector.scalar_tensor_tensor(
                        out=xk_t[:, cb, :], in0=dd, scalar=par["mk"][:, cb:cb + 1],
                        in1=xx_v, op0=OP.mult, op1=OP.add)
                    nc.vector.scalar_tensor_tensor(
                        out=xv_t[:, cb, :], in0=dd, scalar=par["mv"][:, cb:cb + 1],
                        in1=xx_v, op0=OP.mult, op1=OP.add)
                    nc.vector.scalar_tensor_tensor(
                        out=xr_t[:, cb, :], in0=dd, scalar=par["mr"][:, cb:cb + 1],
                        in1=xx_v, op0=OP.mult, op1=OP.add)
                return xk_t, xv_t, xr_t

            xnT = ln_xbar(0, xts, None)
            mix = mix_chunk(xnT)
            xnT_nxt = None
            mix_nxt = None

            for ic in range(NCHA):
                _sid, _ = nc.enter_named_scope(f"A{ic}", False)
                t0 = ic * TCA
                xk_t, xv_t, xr_t = mix

                # Pipelined LN+XBAR+mixes for the NEXT chunk, emitted before
                # this chunk's scan tail so the DVE/ACT/Pool FIFOs don't
                # serialize the next chunk's inputs behind it.
                if ic + 1 < NCHA:
                    xnT_nxt = ln_xbar(ic + 1, xts_nxt, xnT)
                    mix_nxt = mix_chunk(xnT_nxt)
                    xts_prev = xts
                    xts = xts_nxt
                    if ic + 2 < NCHA:
                        xts_nxt = load_x(ic + 2)
                else:
                    xts_prev = xts

                rw_t = wkvp.tile([128, CB, TCA], bf16, name="rw_t", tag="rw")
                for db in range(CB):
                    dsl = slice(db * 128, (db + 1) * 128)
                    psk = psA.tile([128, TCA], f32, name="psk", tag="mmA")
                    for cb in range(CB):
                        nc.tensor.matmul(psk, wk_sb[:, cb, dsl], xk_t[:, cb, :],
                                         start=(cb == 0), stop=(cb == CB - 1))
                    # Two exp flavors: ek = e^k feeds the scans; eku = e^{k+u}
                    # is the instantaneous term in num/den (u = time_first,
                    # folded into the bias host-side: bku = bk + u).
                    ek = wkvp.tile([128, TCA], bf16, name="ek", tag="ek")
                    nc.scalar.activation(out=ek, in_=psk, func=AF.Exp,
                                         bias=par["bk"][:, db:db + 1])
                    eku = wkvp.tile([128, TCA], bf16, name="eku", tag="eku")
                    nc.scalar.activation(out=eku, in_=psk, func=AF.Exp,
                                         bias=par["bku"][:, db:db + 1])

                    psv = psA.tile([128, TCA], f32, name="psv", tag="mmA")
                    for cb in range(CB):
                        nc.tensor.matmul(psv, wv_sb[:, cb, dsl], xv_t[:, cb, :],
                                         start=(cb == 0), stop=(cb == CB - 1))
                    vlin = wkvp.tile([128, TCA], bf16, name="vlin", tag="vlin")
                    nc.scalar.activation(out=vlin, in_=psv, func=AF.Identity,
                                         bias=par["bv"][:, db:db + 1])
                    ekv = wkvp.tile([128, TCA], bf16, name="ekv", tag="ekv")
                    nc.vector.tensor_mul(ekv, vlin, ek)
                    ekvu = wkvp.tile([128, TCA], bf16, name="ekvu", tag="ekvu")
                    nc.vector.tensor_mul(ekvu, vlin, eku)

                    psr = psA.tile([128, TCA], f32, name="psr", tag="mmA")
                    for cb in range(CB):
                        nc.tensor.matmul(psr, wr_sb[:, cb, dsl], xr_t[:, cb, :],
                                         start=(cb == 0), stop=(cb == CB - 1))
                    # sigmoid(r)*num/den == num / (den*(1+exp(-r))) — avoids
                    # the Sigmoid table set; everything stays in ln/exp.
                    er = wkvp.tile([128, TCA], bf16, name="er", tag="sr")
                    nc.scalar.activation(out=er, in_=psr, func=AF.Exp, scale=-1.0,
                                         bias=par["brn"][:, db:db + 1])

                    lam_s = par["lam"][:, db:db + 1]
                    # A and B scans share one padded tile so the carry
                    # copy-in/copy-out is a single [128, 2] ACT op each.
                    ABpad = wkvp.tile([128, 2, TCA + 1], bf16, name="ABpad",
                                      tag="ABpad")
                    if ic == 0:
                        nc.vector.memset(ABpad[:, :, 0], 0.0)
                        a_init = 0.0
                        b_init = 0.0
                    else:
                        nc.scalar.copy(out=ABpad[:, :, 0],
                                       in_=ABcar[:, 2 * db:2 * db + 2])
                        a_init = ABcar[:, 2 * db:2 * db + 1]
                        b_init = ABcar[:, 2 * db + 1:2 * db + 2]
                    nc.vector.tensor_tensor_scan(
                        out=ABpad[:, 0, 1:TCA + 1],
                        data0=lam_s.broadcast_to([128, TCA]),
                        data1=ekv, initial=a_init, op0=OP.mult, op1=OP.add)
                    nc.vector.tensor_tensor_scan(
                        out=ABpad[:, 1, 1:TCA + 1],
                        data0=lam_s.broadcast_to([128, TCA]),
                        data1=ek, initial=b_init, op0=OP.mult, op1=OP.add)
                    if ic < NCHA - 1:
                        nc.scalar.copy(out=ABcar[:, 2 * db:2 * db + 2],
                                       in_=ABpad[:, :, TCA])

                    # Tail (all-bf16 tensor_tensor ops hit the DVE 2x mode):
                    #   num -> ekvu (+= A_prev), den -> eku (+= B_prev)
                    #   1/(den*(1+er)) = exp(-(ln(1+er) + ln(den)))
                    nc.vector.tensor_add(ekvu, ekvu, ABpad[:, 0, 0:TCA])
                    nc.vector.tensor_add(eku, eku, ABpad[:, 1, 0:TCA])
                    l1 = wkvp.tile([128, TCA], f32, name="l1", tag="l1")
                    nc.scalar.activation(out=l1, in_=er, func=AF.Ln, bias=one_t)
                    lden = wkvp.tile([128, TCA], f32, name="lden", tag="lden")
                    nc.scalar.activation(out=lden, in_=eku, func=AF.Ln)
                    nc.vector.tensor_add(l1, l1, lden)
                    rinv = wkvp.tile([128, TCA], bf16, name="rinv", tag="rinv")
                    nc.scalar.activation(out=rinv, in_=l1, func=AF.Exp,
                                         scale=-1.0)
                    nc.vector.tensor_mul(rw_t[:, db, :], ekvu, rinv)

                # Wo back to token-major + residual
                for tt in range(NTT):
                    for chh in range(2):
                        pso = psO.tile([128, 512], f32, name="pso", tag="mmO")
                        for db in range(CB):
                            nc.tensor.matmul(
                                pso,
                                rw_t[:, db, tt * 128:(tt + 1) * 128],
                                wo_sb[:, db, chh * 512:(chh + 1) * 512],
                                start=(db == 0), stop=(db == CB - 1))
                        x2sb = x2p.tile([128, 512], bf16, name="x2sb", tag="x2")
                        nc.vector.tensor_add(
                            x2sb, xts_prev[tt][:, chh * 512:(chh + 1) * 512], pso)
                        jb = ic * 2 + tt // 2
                        row = (tt % 2) * 128
                        nc.sync.dma_start(
                            out=x2_dram[jb][row:row + 128,
                                            chh * 512:(chh + 1) * 512],
                            in_=x2sb)
                xnT = xnT_nxt
                mix = mix_nxt
                nc.leave_named_scope(f"A{ic}", _sid, False)

        # ================= Phase B: ChannelMix =================
        with ExitStack() as ph:
            x2tp = ph.enter_context(tc.tile_pool(name="x2tp", bufs=5))
            statsp = ph.enter_context(tc.tile_pool(name="statsB", bufs=4))
            xnp = ph.enter_context(tc.tile_pool(name="xnpB", bufs=2))
            xnTp = ph.enter_context(tc.tile_pool(name="xnTB", bufs=2))
            mixp = ph.enter_context(tc.tile_pool(name="mixB", bufs=1))
            ffp = ph.enter_context(tc.tile_pool(name="ffp", bufs=2))
            wp = ph.enter_context(tc.tile_pool(name="wB", bufs=1))
            psKV = ph.enter_context(tc.tile_pool(name="psKV", bufs=1, space="PSUM"))
            psKF = ph.enter_context(tc.tile_pool(name="psKF", bufs=1, space="PSUM"))
            psSm = ph.enter_context(tc.tile_pool(name="psSm", bufs=2, space="PSUM"))

            NTT = TCB // 128  # 2
            PD = 16

            def load_x2(jc):
                x2ts = []
                for tt in range(NTT):
                    x2t = x2tp.tile([128, C], bf16, name="x2t", tag="x2t")
                    eng = nc.sync if tt % 2 == 0 else nc.scalar
                    eng.dma_start(
                        out=x2t,
                        in_=x2_dram[jc][tt * 128:(tt + 1) * 128, :])
                    x2ts.append(x2t)
                return x2ts

            def ln_xbar2(jc, x2ts, prev_xnT):
                xnT = xnTp.tile([128, CB, TCB + PD], bf16, name="xnT2",
                                tag="xnT2")
                if jc == 0:
                    nc.vector.memset(xnT[:, :, PD - 1], 0.0)
                else:
                    nc.scalar.copy(out=xnT[:, :, PD - 1],
                                   in_=prev_xnT[:, :, PD - 1 + TCB])
                for tt in range(NTT):
                    xnh = ln_apply(x2ts[tt], statsp, xnp)
                    nc.sync.dma_start(
                        out=xnT[:, :, PD + tt * 128: PD + (tt + 1) * 128],
                        in_=xnh, transpose=True)
                return xnT

            x2ts = load_x2(0)
            x2ts_nxt = load_x2(1)
            if with_b2r:
                b2r_bcast = bass.AP(tensor=b2r_d.tensor, offset=b2r_d.offset,
                                    ap=[[0, 128]] + list(b2r_d.ap))
                nc.sync.dma_start(out=b2r_t, in_=b2r_bcast)

            # Weights streamed in use-order on the ACT hwdge queue: wkf
            # (f-quarter-major host layout so each quarter is one descriptor
            # per partition AND kf can start after the first lands), wrf,
            # then wvf under the kf MMs.
            wkf_sb = wp.tile([128, 4, CB, FFN // 4], bf16)
            wrf_sb = wp.tile([128, CB, C], bf16)
            wvf_sb = wp.tile([128, FB, C], bf16)
            for q in range(4):
                nc.gpsimd.dma_start(out=wkf_sb[:, q], in_=wkf_d[:, q])
                if q == 0:
                    nc.gpsimd.dma_start(out=wrf_sb, in_=wrf_d)
            for q in range(4):
                nc.gpsimd.dma_start(
                    out=wvf_sb[:, q * (FB // 4):(q + 1) * (FB // 4), :],
                    in_=wvf_d[:, q * (FB // 4):(q + 1) * (FB // 4), :])

            def mix2_chunk(xnT):
                xk_t = mixp.tile([128, CB, TCB], bf16, name="xk2_t", tag="xk2")
                xr_t = mixp.tile([128, CB, TCB], bf16, name="xr2_t", tag="xr2")
                for cb in range(CB):
                    xn_v = xnT[:, cb, PD:PD + TCB]
                    xx_v = xnT[:, cb, PD - 1:PD - 1 + TCB]
                    dd = mixp.tile([128, TCB], bf16, name="dd2", tag="dd2", bufs=1)
                    nc.vector.tensor_sub(dd, xn_v, xx_v)
                    nc.vector.scalar_tensor_tensor(
                        out=xk_t[:, cb, :], in0=dd, scalar=par["mfk"][:, cb:cb + 1],
                        in1=xx_v, op0=OP.mult, op1=OP.add)
                    nc.vector.scalar_tensor_tensor(
                        out=xr_t[:, cb, :], in0=dd, scalar=par["mfr"][:, cb:cb + 1],
                        in1=xx_v, op0=OP.mult, op1=OP.add)
                return xk_t, xr_t

            xnT = ln_xbar2(0, x2ts, None)
            mix = mix2_chunk(xnT)
            xnT_nxt = None
            mix_nxt = None

            for jc in range(NCHB):
                _sid, _ = nc.enter_named_scope(f"B{jc}", False)
                t0 = jc * TCB
                xk_t, xr_t = mix

                # Pipelined LN+XBAR+mixes for the next chunk (same reasoning
                # as phase A: keep the DVE/ACT FIFOs from serializing the
                # next chunk's inputs behind this chunk's elementwise tail).
                if jc + 1 < NCHB:
                    xnT_nxt = ln_xbar2(jc + 1, x2ts_nxt, xnT)
                    mix_nxt = mix2_chunk(xnT_nxt)
                    x2ts_prev = x2ts
                    x2ts = x2ts_nxt
                    if jc + 2 < NCHB:
                        x2ts_nxt = load_x2(jc + 2)
                else:
                    x2ts_prev = x2ts

                # kf for all 32 f-blocks first (one long PE run), then kv as
                # sequential psum groups. kfs is held in two half tiles with
                # bufs=3 (1.5-buffering): the next chunk's kf half-0 can start
                # as soon as this chunk's kv has consumed fb 0..15.
                FBH = FB // 2
                kfs_h = [ffp.tile([128, FBH, TCB], bf16, name=f"kfs{h}",
                                  tag="kfs", bufs=3) for h in range(2)]
                for fb in range(FB):
                    pskf = psKF.tile([128, TCB], f32, name="pskf", tag="kfps",
                                     bufs=4)
                    for cb in range(CB):
                        nc.tensor.matmul(
                            pskf,
                            wkf_sb[:, fb // 8, cb,
                                   (fb % 8) * 128:(fb % 8 + 1) * 128],
                            xk_t[:, cb, :], start=(cb == 0), stop=(cb == CB - 1))
                    kfb = ffp.tile([128, TCB], bf16, name="kfb", tag="kf", bufs=2)
                    nc.scalar.activation(out=kfb, in_=pskf, func=AF.Relu,
                                         bias=bkf_t[:, fb:fb + 1])
                    nc.vector.tensor_mul(kfs_h[fb // FBH][:, fb % FBH, :],
                                         kfb, kfb)

                for tt in range(NTT):
                    for chh in range(2):
                        kvp = psKV.tile([128, 512], f32, name="kvp", tag="kv",
                                        bufs=2)
                        for fb in range(FB):
                            nc.tensor.matmul(
                                kvp,
                                kfs_h[fb // FBH][:, fb % FBH,
                                                 tt * 128:(tt + 1) * 128],
                                wvf_sb[:, fb, chh * 512:(chh + 1) * 512],
                                start=(fb == 0), stop=(fb == FB - 1))
                        psr2 = psSm.tile([128, 512], f32, name="psr2", tag="sm")
                        for cb in range(CB):
                            nc.tensor.matmul(
                                psr2,
                                xr_t[:, cb, tt * 128:(tt + 1) * 128],
                                wrf_sb[:, cb, chh * 512:(chh + 1) * 512],
                                start=(cb == 0), stop=(cb == CB - 1))
                        # sigmoid via exp/ln only (same ACT table set):
                        # sig = exp(-ln(1+exp(-r)))
                        if with_b2r:
                            nc.vector.tensor_add(
                                psr2, psr2,
                                b2r_t[:, chh * 512:(chh + 1) * 512])
                        er2 = ffp.tile([128, 512], bf16, name="er2", tag="sr2")
                        nc.scalar.activation(out=er2, in_=psr2, func=AF.Exp,
                                             scale=-1.0)
                        nc.scalar.activation(out=er2, in_=er2, func=AF.Ln,
                                             bias=one_t)
                        nc.scalar.activation(out=er2, in_=er2, func=AF.Exp,
                                             scale=-1.0)
                        tmp = ffp.tile([128, 512], f32, name="tmp", tag="tmp", bufs=2)
                        nc.vector.tensor_mul(tmp, er2, kvp)
                        nc.vector.tensor_add(
                            tmp, tmp,
                            x2ts_prev[tt][:, chh * 512:(chh + 1) * 512])
                        nc.sync.dma_start(
                            out=y_d[t0 + tt * 128: t0 + (tt + 1) * 128,
                                    chh * 512:(chh + 1) * 512],
                            in_=tmp)
                xnT = xnT_nxt
                mix = mix_nxt
                nc.leave_named_scope(f"B{jc}", _sid, False)

    nc.compile()
    return nc


def get_program(with_b2r=False):
    key = ("nc", with_b2r)
    if key not in _CACHE:
        _CACHE[key] = _build_program(with_b2r)
    return _CACHE[key]


def host_inputs(inputs):
    """Host-side precompute: transposed bf16 weights + derived per-channel
    vectors. Returns the shared (per-core-identical) part of the in_map."""
    bf = ml_dtypes.bfloat16
    f32 = np.float32

    def v(name):
        return np.asarray(inputs[name], f32).reshape(-1)

    g1, b1 = v("ln1_g"), v("ln1_b")
    g2, b2 = v("ln2_g"), v("ln2_b")
    Wk = np.asarray(inputs["Wk"], f32)
    Wv = np.asarray(inputs["Wv"], f32)
    Wr = np.asarray(inputs["Wr"], f32)
    Wkf = np.asarray(inputs["Wk_ffn"], f32)
    Wrf = np.asarray(inputs["Wr_ffn"], f32)

    # The XBAR transpose writes standard channel blocking: transposed row
    # c = cb*128 + p lands at [partition p, block cb] — same layout as the
    # weight DMAs, so no permutation is needed anywhere.
    def tb(a):
        # [in_ch, cols] -> SBUF tile layout [128, in_blocks, cols]
        # (partition-contiguous so the DMA is 1 descriptor per partition)
        n, m = a.shape
        return np.ascontiguousarray(
            a.reshape(n // 128, 128, m).transpose(1, 0, 2).astype(bf))

    def col(x):  # channel panel [128, n]: entry [p, a] = x[a*128 + p]
        return np.ascontiguousarray(x.reshape(-1, 128).T)

    pars = np.concatenate([
        col(v("att_mix_k")), col(v("att_mix_v")), col(v("att_mix_r")),
        col(v("ffn_mix_k")), col(v("ffn_mix_r")),
        col(np.exp(-np.exp(v("time_decay"))).astype(f32)),
        col((Wk @ b1).astype(f32)),
        # u = time_first folded into the bias of the second exp flavor:
        # eku = exp(psk + Wk@b1 + u)
        col((Wk @ b1 + v("time_first")).astype(f32)),
        col((Wv @ b1).astype(f32)),
        col((-(Wr @ b1)).astype(f32)),
        col((Wkf @ b2).astype(f32)),
    ], axis=1).astype(f32)
    assert pars.shape == (128, NPARC)

    shared = {
        # LN gain folded in: row c of W.T scaled by g[c]
        "wkT": tb(Wk.T * g1[:, None]),
        "wvT": tb(Wv.T * g1[:, None]),
        "wrT": tb(Wr.T * g1[:, None]),
        "woT": tb(np.asarray(inputs["Wo"], f32).T),
        # wkf additionally split f-quarter-major: [128, 4, CB, FFN//4]
        "wkfT": np.ascontiguousarray(
            tb(Wkf.T * g2[:, None]).reshape(128, CB, 4, FFN // 4)
            .transpose(0, 2, 1, 3)),
        "wvfT": tb(np.asarray(inputs["Wv_ffn"], f32).T),
        "wrfT": tb(Wrf.T * g2[:, None]),
        "pars": np.ascontiguousarray(pars),
    }
    b2r = (Wrf @ b2).astype(f32)
    with_b2r = bool(np.any(b2r != 0.0))
    if with_b2r:
        shared["b2r"] = b2r
    return shared, with_b2r


def _ensure_axon_hooks():
    """The agent image's antenv lacks axon_hooks; bass_utils trace=True needs
    it. Install a shim wired to the injected libaxon_pjrt.so NTFF profiler."""
    try:
        import antenv.axon_hooks  # noqa: F401
        return
    except ImportError:
        pass
    import types
    mod = types.ModuleType("antenv.axon_hooks")
    mod._hook = None

    def set_axon_ntff_profile_hook(h):
        mod._hook = h

    def get_axon_ntff_profile_hook():
        return mod._hook

    mod.set_axon_ntff_profile_hook = set_axon_ntff_profile_hook
    mod.get_axon_ntff_profile_hook = get_axon_ntff_profile_hook
    sys.modules["antenv.axon_hooks"] = mod
    import antenv
    antenv.axon_hooks = mod
    try:
        from trn_agent_boot.trn_boot import _ntff_profile_via_ctypes
        so = "/opt/axon/libaxon_pjrt.so"
        if os.path.exists(so):
            mod._hook = _ntff_profile_via_ctypes(so)
    except Exception as e:  # pragma: no cover - degrade to no tracing
        print("ntff hook unavailable:", e)


def kernel(**inputs):
    from concourse import bass_utils

    shared, with_b2r = host_inputs(inputs)
    nc = get_program(with_b2r)
    X = np.asarray(inputs["x"], np.float32).astype(ml_dtypes.bfloat16)
    in_maps = [dict(shared, x=np.ascontiguousarray(X[b])) for b in range(NCORES)]

    trace = os.environ.get("KERNEL_TRACE", "0") == "1"
    if trace:
        _ensure_axon_hooks()
    res = bass_utils.run_bass_kernel_spmd(
        nc, in_maps, core_ids=list(range(NCORES)), trace=trace)
    kernel._last_exec_ns = res.exec_time_ns
    kernel._last_results = res
    out = np.stack([r["y"] for r in res.results], axis=0)
    return out


kernel._last_exec_ns = None


# revision 59
# speedup vs baseline: 1.5186x; 1.0093x over previous
"""RWKV-4 block (TimeMix + ChannelMix) Trainium2 Bass kernel.

Full inputs in, full outputs out. Sharding: data-parallel over batch B=8
across the 8 NeuronCores (one batch element per core, no collectives).

Per-core layout strategy:
  - LayerNorm + stats in token-major [128 tokens, C] (bn_stats along free
    dim), then XBAR DMA-transpose to feature-major [128 chan, T] in one op
    per token tile. Channels are interleaved across partitions
    (channel c = p*8 + cb lives at [partition p, block cb]) so the XBAR's
    row-write order matches the [p, cb, t] tile layout; weight rows and
    per-channel mix params are permuted host-side to match.
  - token_shift is a free-dim offset view of a front-padded feature-major
    tile; the pad column is copied from the previous chunk's last column.
  - k/v/r and FFN matmuls in bf16 (fp32 PSUM accumulation).
  - WKV recurrence A_t = lam*A_{t-1} + e^{k_t+u} v_t (and B with e^{k_t+u})
    runs as a native DVE tensor_tensor_scan per 128-channel block, fp32,
    chained across T-chunks via the `initial` operand (prev scan's last
    element). e^u is folded into the exp bias host-side: it cancels in
    y = num/den, so num = ekv + A_prev and den = ek + B_prev are plain adds.
    Division via ACT Ln+Exp (1/x = exp(-ln x), denominator positive).
  - Elementwise work is spread across engines: DVE (scans, adds, k-mix),
    GpSimd (v/r mixes, subs, rw mult), ACT (exp/ln, psv+bv via Copy bias).
"""

import os
import sys

import numpy as np

if "/opt/trn_rl_repo" not in sys.path:
    sys.path.insert(0, "/opt/trn_rl_repo")

import ml_dtypes

B, T, C, FFN = 8, 2048, 1024, 4096
NCORES = 8
CB = C // 128      # 8 channel blocks
FB = FFN // 128    # 32 ffn blocks
TCA = 512          # TimeMix chunk (tokens)
NCHA = T // TCA    # 4
TCB = 256          # FFN chunk (tokens)
NCHB = T // TCB    # 8
LN_EPS = 1e-5

_CACHE = {}

# pars packing: 10 per-C params as [128, 8] panels + bkf as [128, 32]
PAR_NAMES = ["mk", "mv", "mr", "mfk", "mfr", "lam", "bk", "bku", "bv", "brn"]
NPARC = len(PAR_NAMES) * CB + FB  # 112


def _build_program(with_b2r=False):
    import concourse.bass as bass  # noqa: F401
    from concourse import bacc
    import concourse.tile as tile
    import concourse.mybir as mybir
    from contextlib import ExitStack

    f32 = mybir.dt.float32
    bf16 = mybir.dt.bfloat16
    AF = mybir.ActivationFunctionType
    OP = mybir.AluOpType

    # Pin ALL activations to the one table set that contains every function
    # we use (ln/exp/relu/copy). Without this the table-load pass picks the
    # first set containing each function and thrashes ACT_TABLE_LOADs
    # (~2.7us each).
    import concourse.hw_specs as hw_specs
    if not getattr(hw_specs, "_rwkv_patched", False):
        _orig_gat = hw_specs.get_activation_tables

        def _only_lnexp(arch):
            t = _orig_gat(arch)
            keep = "natural_log_exp_and_others"
            return {name: (funcs if name == keep else set())
                    for name, funcs in t.items()}

        hw_specs.get_activation_tables = _only_lnexp
        bacc.get_activation_tables = _only_lnexp
        hw_specs._rwkv_patched = True

    nc = bacc.Bacc("TRN2", target_bir_lowering=False, debug=False,
                   enable_asserts=False)

    # ---------------- DRAM I/O ----------------
    x_d = nc.dram_tensor("x", [T, C], bf16, kind="ExternalInput").ap()
    y_d = nc.dram_tensor("y", [T, C], f32, kind="ExternalOutput").ap()

    # Weights are pre-arranged host-side to the SBUF tile layout
    # [128, blocks, cols] (partition-contiguous) so each weight DMA needs
    # only one descriptor per partition — descriptor generation for the
    # naive strided layout serialized the SP queue for ~6us per transfer.
    wk_d = nc.dram_tensor("wkT", [128, CB, C], bf16, kind="ExternalInput").ap()
    wv_d = nc.dram_tensor("wvT", [128, CB, C], bf16, kind="ExternalInput").ap()
    wr_d = nc.dram_tensor("wrT", [128, CB, C], bf16, kind="ExternalInput").ap()
    wo_d = nc.dram_tensor("woT", [128, CB, C], bf16, kind="ExternalInput").ap()
    wkf_d = nc.dram_tensor("wkfT", [128, 4, CB, FFN // 4], bf16,
                           kind="ExternalInput").ap()
    wvf_d = nc.dram_tensor("wvfT", [128, FB, C], bf16, kind="ExternalInput").ap()
    wrf_d = nc.dram_tensor("wrfT", [128, CB, C], bf16, kind="ExternalInput").ap()

    # All small per-channel params packed into ONE [128, NPARC] fp32 tensor
    # (a single contiguous DMA instead of ~11 tiny strided ones).
    pars_d = nc.dram_tensor("pars", [128, NPARC], f32, kind="ExternalInput").ap()
    if with_b2r:
        b2r_d = nc.dram_tensor("b2r", [C], f32, kind="ExternalInput").ap()

    with tile.TileContext(nc) as tc, ExitStack() as glob:
        const = glob.enter_context(tc.tile_pool(name="const", bufs=1))
        dram = glob.enter_context(tc.tile_pool(name="dram", bufs=1, space="DRAM"))

        pars_t = const.tile([128, NPARC], f32)
        par = {}
        for i, p in enumerate(PAR_NAMES):
            par[p] = pars_t[:, i * CB:(i + 1) * CB]
        bkf_t = pars_t[:, len(PAR_NAMES) * CB:len(PAR_NAMES) * CB + FB]
        if with_b2r:
            b2r_t = const.tile([128, C], f32)

        eps_t = const.tile([128, 1], f32)
        one_t = const.tile([128, 1], f32)

        # x2 is split per B-chunk so phase B's reads only depend on the
        # A-chunk that produced those tokens (whole-tile DRAM deps would
        # stall B0 until all of A finished).
        x2_dram = [dram.tile([TCB, C], bf16, name=f"x2d{j}")
                   for j in range(NCHB)]

        # ---- helper: LN stats + apply for one [128, C] token tile ----
        def ln_apply(xt, statsp, xnp):
            st6 = statsp.tile([128, 2, 6], f32, name="st6", tag="st6")
            nc.vector.bn_stats(out=st6[:, 0, :], in_=xt[:, 0:512])
            nc.vector.bn_stats(out=st6[:, 1, :], in_=xt[:, 512:1024])
            mv_t = statsp.tile([128, 2], f32, name="mv_t", tag="mv")
            nc.vector.bn_aggr(out=mv_t, in_=st6)
            # rstd = exp(-0.5*ln(var+eps)) — keeps ACT inside the ln/exp
            # table set (Sqrt lives in a different set -> ~2.7us reload).
            ldv = statsp.tile([128, 1], f32, name="ldv", tag="ldv")
            nc.scalar.activation(out=ldv, in_=mv_t[:, 1:2], func=AF.Ln,
                                 bias=eps_t)
            rstd = statsp.tile([128, 1], f32, name="rstd", tag="rstd")
            nc.scalar.activation(out=rstd, in_=ldv, func=AF.Exp, scale=-0.5)
            xnh = xnp.tile([128, C], bf16, name="xnh", tag="xnh")
            nc.vector.tensor_scalar(out=xnh, in0=xt, scalar1=mv_t[:, 0:1],
                                    scalar2=rstd, op0=OP.subtract, op1=OP.mult)
            return xnh

        # ================= Phase A: TimeMix =================
        with ExitStack() as ph:
            xtp = ph.enter_context(tc.tile_pool(name="xtp", bufs=12))
            carp = ph.enter_context(tc.tile_pool(name="carp", bufs=1))
            statsp = ph.enter_context(tc.tile_pool(name="statsA", bufs=4))
            xnp = ph.enter_context(tc.tile_pool(name="xnpA", bufs=2))
            xnTp = ph.enter_context(tc.tile_pool(name="xnTA", bufs=2))
            mixp = ph.enter_context(tc.tile_pool(name="mixA", bufs=2))
            wkvp = ph.enter_context(tc.tile_pool(name="wkv", bufs=2))
            x2p = ph.enter_context(tc.tile_pool(name="x2p", bufs=2))
            wp = ph.enter_context(tc.tile_pool(name="wA", bufs=1))
            psA = ph.enter_context(tc.tile_pool(name="psA", bufs=5, space="PSUM"))
            psO = ph.enter_context(tc.tile_pool(name="psO", bufs=2, space="PSUM"))

            NTT = TCA // 128  # 4 token tiles per chunk

            # First chunk's x tiles + params first so compute starts ASAP,
            # then weights in use-order (wk -> wv -> wr -> wo), one big DMA
            # each.
            PD = 16

            def load_x(ic):
                # alternate between the two hwdge queues so tile loads are
                # not serialized behind each other or the XBARs
                xts = []
                for tt in range(NTT):
                    xt = xtp.tile([128, C], bf16, name="xt", tag="xt")
                    eng = nc.sync if tt % 2 == 0 else nc.scalar
                    eng.dma_start(
                        out=xt,
                        in_=x_d[ic * TCA + tt * 128: ic * TCA + (tt + 1) * 128, :])
                    xts.append(xt)
                return xts

            def ln_xbar(ic, xts, prev_xnT):
                # feature-major, front-padded for token_shift. The XBAR
                # transpose needs a 32B-aligned destination, so data starts
                # at column PD=16 and the shift-pad lives at column 15.
                xnT = xnTp.tile([128, CB, TCA + PD], bf16, name="xnT", tag="xnT")
                if ic == 0:
                    nc.vector.memset(xnT[:, :, PD - 1], 0.0)
                else:
                    nc.scalar.copy(out=xnT[:, :, PD - 1],
                                   in_=prev_xnT[:, :, PD - 1 + TCA])
                for tt in range(NTT):
                    xnh = ln_apply(xts[tt], statsp, xnp)
                    # XBAR transpose [128 tok, C] -> [128 chan, cb, 128 tok]
                    nc.sync.dma_start(
                        out=xnT[:, :, PD + tt * 128: PD + (tt + 1) * 128],
                        in_=xnh, transpose=True)
                return xnT

            xts = load_x(0)
            xts_nxt = load_x(1)
            nc.sync.dma_start(out=pars_t, in_=pars_d)
            nc.vector.memset(eps_t, LN_EPS)
            nc.vector.memset(one_t, 1.0)

            # Bulk weight DMAs go through the ACT hwdge queue so the
            # latency-critical XBAR transposes on the SP queue are not stuck
            # behind 8MB of weight traffic.
            wk_sb = wp.tile([128, CB, C], bf16)
            wv_sb = wp.tile([128, CB, C], bf16)
            wr_sb = wp.tile([128, CB, C], bf16)
            wo_sb = wp.tile([128, CB, C], bf16)
            for w_sb, w_d in ((wk_sb, wk_d), (wv_sb, wv_d),
                              (wr_sb, wr_d), (wo_sb, wo_d)):
                nc.gpsimd.dma_start(out=w_sb, in_=w_d)

            # Cross-chunk scan carries live in a persistent [128, 2*CB] tile
            # (ABpad buffers rotate within a chunk, so a prev-chunk tile
            # reference would be recycled before the next chunk reads it).
            # Layout: [:, 2*db] = A carry, [:, 2*db+1] = B carry.
            ABcar = carp.tile([128, 2 * CB], f32)

            def mix_chunk(xnT):
                # xk on DVE (single stt); xv/xr on the otherwise-idle GpSimd
                # as mul+add pairs — the DVE is phase A's long pole and the
                # mixes are pipelined a full chunk ahead so GpSimd's latency
                # is hidden under the previous chunk's matmuls.
                xk_t = mixp.tile([128, CB, TCA], bf16, name="xk_t", tag="xk")
                xv_t = mixp.tile([128, CB, TCA], bf16, name="xv_t", tag="xv")
                xr_t = mixp.tile([128, CB, TCA], bf16, name="xr_t", tag="xr")
                for cb in range(CB):
                    xn_v = xnT[:, cb, PD:PD + TCA]          # xn[t]
                    xx_v = xnT[:, cb, PD - 1:PD - 1 + TCA]  # xn[t-1]
                    dd = mixp.tile([128, TCA], bf16, name="dd", tag="dd", bufs=2)
                    nc.vector.tensor_sub(dd, xn_v, xx_v)
                    nc.vector.scalar_tensor_tensor(
                        out=xk_t[:, cb, :], in0=dd, scalar=par["mk"][:, cb:cb + 1],
                        in1=xx_v, op0=OP.mult, op1=OP.add)
                    nc.vector.scalar_tensor_tensor(
                        out=xv_t[:, cb, :], in0=dd, scalar=par["mv"][:, cb:cb + 1],
                        in1=xx_v, op0=OP.mult, op1=OP.add)
                    nc.vector.scalar_tensor_tensor(
                        out=xr_t[:, cb, :], in0=dd, scalar=par["mr"][:, cb:cb + 1],
                        in1=xx_v, op0=OP.mult, op1=OP.add)
                return xk_t, xv_t, xr_t

            xnT = ln_xbar(0, xts, None)
            mix = mix_chunk(xnT)
            xnT_nxt = None
            mix_nxt = None

            for ic in range(NCHA):
                _sid, _ = nc.enter_named_scope(f"A{ic}", False)
                t0 = ic * TCA
                xk_t, xv_t, xr_t = mix
                xts_prev = xts

                rw_t = wkvp.tile([128, CB, TCA], bf16, name="rw_t", tag="rw")
                for db in range(CB):
                    # Pipelined LN+XBAR+mixes for the NEXT chunk, emitted
                    # after the scan chain's head (db 0-2) so the serial
                    # cross-chunk scan carry is not delayed, but early
                    # enough that the next chunk's matmul inputs are ready
                    # before this chunk's PE stream drains.
                    if db == 3 and ic + 1 < NCHA:
                        xnT_nxt = ln_xbar(ic + 1, xts_nxt, xnT)
                        mix_nxt = mix_chunk(xnT_nxt)
                        xts = xts_nxt
                        if ic + 2 < NCHA:
                            xts_nxt = load_x(ic + 2)
                    dsl = slice(db * 128, (db + 1) * 128)
                    psk = psA.tile([128, TCA], f32, name="psk", tag="mmA")
                    for cb in range(CB):
                        nc.tensor.matmul(psk, wk_sb[:, cb, dsl], xk_t[:, cb, :],
                                         start=(cb == 0), stop=(cb == CB - 1))
                    # Two exp flavors: ek = e^k feeds the scans; eku = e^{k+u}
                    # is the instantaneous term in num/den (u = time_first,
                    # folded into the bias host-side: bku = bk + u).
                    ek = wkvp.tile([128, TCA], bf16, name="ek", tag="ek")
                    nc.scalar.activation(out=ek, in_=psk, func=AF.Exp,
                                         bias=par["bk"][:, db:db + 1])
                    eku = wkvp.tile([128, TCA], bf16, name="eku", tag="eku")
                    nc.scalar.activation(out=eku, in_=psk, func=AF.Exp,
                                         bias=par["bku"][:, db:db + 1])

                    psv = psA.tile([128, TCA], f32, name="psv", tag="mmA")
                    for cb in range(CB):
                        nc.tensor.matmul(psv, wv_sb[:, cb, dsl], xv_t[:, cb, :],
                                         start=(cb == 0), stop=(cb == CB - 1))
                    vlin = wkvp.tile([128, TCA], bf16, name="vlin", tag="vlin")
                    nc.scalar.activation(out=vlin, in_=psv, func=AF.Identity,
                                         bias=par["bv"][:, db:db + 1])
                    ekv = wkvp.tile([128, TCA], bf16, name="ekv", tag="ekv")
                    nc.vector.tensor_mul(ekv, vlin, ek)
                    ekvu = wkvp.tile([128, TCA], bf16, name="ekvu", tag="ekvu")
                    nc.vector.tensor_mul(ekvu, vlin, eku)

                    psr = psA.tile([128, TCA], f32, name="psr", tag="mmA")
                    for cb in range(CB):
                        nc.tensor.matmul(psr, wr_sb[:, cb, dsl], xr_t[:, cb, :],
                                         start=(cb == 0), stop=(cb == CB - 1))
                    # sigmoid(r)*num/den == num / (den*(1+exp(-r))) — avoids
                    # the Sigmoid table set; everything stays in ln/exp.
                    er = wkvp.tile([128, TCA], bf16, name="er", tag="sr")
                    nc.scalar.activation(out=er, in_=psr, func=AF.Exp, scale=-1.0,
                                         bias=par["brn"][:, db:db + 1])

                    lam_s = par["lam"][:, db:db + 1]
                    # A and B scans share one padded tile so the carry
                    # copy-in/copy-out is a single [128, 2] ACT op each.
                    ABpad = wkvp.tile([128, 2, TCA + 1], bf16, name="ABpad",
                                      tag="ABpad")
                    if ic == 0:
                        nc.vector.memset(ABpad[:, :, 0], 0.0)
                        a_init = 0.0
                        b_init = 0.0
                    else:
                        nc.scalar.copy(out=ABpad[:, :, 0],
                                       in_=ABcar[:, 2 * db:2 * db + 2])
                        a_init = ABcar[:, 2 * db:2 * db + 1]
                        b_init = ABcar[:, 2 * db + 1:2 * db + 2]
                    nc.vector.tensor_tensor_scan(
                        out=ABpad[:, 0, 1:TCA + 1],
                        data0=lam_s.broadcast_to([128, TCA]),
                        data1=ekv, initial=a_init, op0=OP.mult, op1=OP.add)
                    nc.vector.tensor_tensor_scan(
                        out=ABpad[:, 1, 1:TCA + 1],
                        data0=lam_s.broadcast_to([128, TCA]),
                        data1=ek, initial=b_init, op0=OP.mult, op1=OP.add)
                    if ic < NCHA - 1:
                        nc.scalar.copy(out=ABcar[:, 2 * db:2 * db + 2],
                                       in_=ABpad[:, :, TCA])

                    # Tail (all-bf16 tensor_tensor ops hit the DVE 2x mode):
                    #   num -> ekvu (+= A_prev), den -> eku (+= B_prev)
                    #   1/(den*(1+er)) = exp(-(ln(1+er) + ln(den)))
                    nc.vector.tensor_add(ekvu, ekvu, ABpad[:, 0, 0:TCA])
                    nc.vector.tensor_add(eku, eku, ABpad[:, 1, 0:TCA])
                    l1 = wkvp.tile([128, TCA], f32, name="l1", tag="l1")
                    nc.scalar.activation(out=l1, in_=er, func=AF.Ln, bias=one_t)
                    lden = wkvp.tile([128, TCA], f32, name="lden", tag="lden")
                    nc.scalar.activation(out=lden, in_=eku, func=AF.Ln)
                    nc.vector.tensor_add(l1, l1, lden)
                    rinv = wkvp.tile([128, TCA], bf16, name="rinv", tag="rinv")
                    nc.scalar.activation(out=rinv, in_=l1, func=AF.Exp,
                                         scale=-1.0)
                    nc.vector.tensor_mul(rw_t[:, db, :], ekvu, rinv)

                # Wo back to token-major + residual
                for tt in range(NTT):
                    for chh in range(2):
                        pso = psO.tile([128, 512], f32, name="pso", tag="mmO")
                        for db in range(CB):
                            nc.tensor.matmul(
                                pso,
                                rw_t[:, db, tt * 128:(tt + 1) * 128],
                                wo_sb[:, db, chh * 512:(chh + 1) * 512],
                                start=(db == 0), stop=(db == CB - 1))
                        x2sb = x2p.tile([128, 512], bf16, name="x2sb", tag="x2")
                        nc.vector.tensor_add(
                            x2sb, xts_prev[tt][:, chh * 512:(chh + 1) * 512], pso)
                        jb = ic * 2 + tt // 2
                        row = (tt % 2) * 128
                        nc.sync.dma_start(
                            out=x2_dram[jb][row:row + 128,
                                            chh * 512:(chh + 1) * 512],
                            in_=x2sb)
                xnT = xnT_nxt
                mix = mix_nxt
                nc.leave_named_scope(f"A{ic}", _sid, False)

        # ================= Phase B: ChannelMix =================
        with ExitStack() as ph:
            x2tp = ph.enter_context(tc.tile_pool(name="x2tp", bufs=5))
            statsp = ph.enter_context(tc.tile_pool(name="statsB", bufs=4))
            xnp = ph.enter_context(tc.tile_pool(name="xnpB", bufs=2))
            xnTp = ph.enter_context(tc.tile_pool(name="xnTB", bufs=2))
            mixp = ph.enter_context(tc.tile_pool(name="mixB", bufs=1))
            ffp = ph.enter_context(tc.tile_pool(name="ffp", bufs=2))
            wp = ph.enter_context(tc.tile_pool(name="wB", bufs=1))
            psKV = ph.enter_context(tc.tile_pool(name="psKV", bufs=1, space="PSUM"))
            psKF = ph.enter_context(tc.tile_pool(name="psKF", bufs=1, space="PSUM"))
            psSm = ph.enter_context(tc.tile_pool(name="psSm", bufs=2, space="PSUM"))

            NTT = TCB // 128  # 2
            PD = 16

            def load_x2(jc):
                x2ts = []
                for tt in range(NTT):
                    x2t = x2tp.tile([128, C], bf16, name="x2t", tag="x2t")
                    eng = nc.sync if tt % 2 == 0 else nc.scalar
                    eng.dma_start(
                        out=x2t,
                        in_=x2_dram[jc][tt * 128:(tt + 1) * 128, :])
                    x2ts.append(x2t)
                return x2ts

            def ln_xbar2(jc, x2ts, prev_xnT):
                xnT = xnTp.tile([128, CB, TCB + PD], bf16, name="xnT2",
                                tag="xnT2")
                if jc == 0:
                    nc.vector.memset(xnT[:, :, PD - 1], 0.0)
                else:
                    nc.scalar.copy(out=xnT[:, :, PD - 1],
                                   in_=prev_xnT[:, :, PD - 1 + TCB])
                for tt in range(NTT):
                    xnh = ln_apply(x2ts[tt], statsp, xnp)
                    nc.sync.dma_start(
                        out=xnT[:, :, PD + tt * 128: PD + (tt + 1) * 128],
                        in_=xnh, transpose=True)
                return xnT

            x2ts = load_x2(0)
            x2ts_nxt = load_x2(1)
            if with_b2r:
                b2r_bcast = bass.AP(tensor=b2r_d.tensor, offset=b2r_d.offset,
                                    ap=[[0, 128]] + list(b2r_d.ap))
                nc.sync.dma_start(out=b2r_t, in_=b2r_bcast)

            # Weights streamed in use-order on the ACT hwdge queue: wkf
            # (f-quarter-major host layout so each quarter is one descriptor
            # per partition AND kf can start after the first lands), wrf,
            # then wvf under the kf MMs.
            wkf_sb = wp.tile([128, 4, CB, FFN // 4], bf16)
            wrf_sb = wp.tile([128, CB, C], bf16)
            wvf_sb = wp.tile([128, FB, C], bf16)
            for q in range(4):
                nc.gpsimd.dma_start(out=wkf_sb[:, q], in_=wkf_d[:, q])
                if q == 0:
                    nc.gpsimd.dma_start(out=wrf_sb, in_=wrf_d)
            for q in range(4):
                nc.gpsimd.dma_start(
                    out=wvf_sb[:, q * (FB // 4):(q + 1) * (FB // 4), :],
                    in_=wvf_d[:, q * (FB // 4):(q + 1) * (FB // 4), :])

            def mix2_chunk(xnT):
                xk_t = mixp.tile([128, CB, TCB], bf16, name="xk2_t", tag="xk2")
                xr_t = mixp.tile([128, CB, TCB], bf16, name="xr2_t", tag="xr2")
                for cb in range(CB):
                    xn_v = xnT[:, cb, PD:PD + TCB]
                    xx_v = xnT[:, cb, PD - 1:PD - 1 + TCB]
                    dd = mixp.tile([128, TCB], bf16, name="dd2", tag="dd2", bufs=1)
                    nc.vector.tensor_sub(dd, xn_v, xx_v)
                    nc.vector.scalar_tensor_tensor(
                        out=xk_t[:, cb, :], in0=dd, scalar=par["mfk"][:, cb:cb + 1],
                        in1=xx_v, op0=OP.mult, op1=OP.add)
                    nc.vector.scalar_tensor_tensor(
                        out=xr_t[:, cb, :], in0=dd, scalar=par["mfr"][:, cb:cb + 1],
                        in1=xx_v, op0=OP.mult, op1=OP.add)
                return xk_t, xr_t

            xnT = ln_xbar2(0, x2ts, None)
            mix = mix2_chunk(xnT)
            xnT_nxt = None
            mix_nxt = None

            for jc in range(NCHB):
                _sid, _ = nc.enter_named_scope(f"B{jc}", False)
                t0 = jc * TCB
                xk_t, xr_t = mix

                # Pipelined LN+XBAR+mixes for the next chunk (same reasoning
                # as phase A: keep the DVE/ACT FIFOs from serializing the
                # next chunk's inputs behind this chunk's elementwise tail).
                if jc + 1 < NCHB:
                    xnT_nxt = ln_xbar2(jc + 1, x2ts_nxt, xnT)
                    mix_nxt = mix2_chunk(xnT_nxt)
                    x2ts_prev = x2ts
                    x2ts = x2ts_nxt
                    if jc + 2 < NCHB:
                        x2ts_nxt = load_x2(jc + 2)
                else:
                    x2ts_prev = x2ts

                # kf for all 32 f-blocks first (one long PE run), then kv as
                # sequential psum groups. kfs is held in two half tiles with
                # bufs=3 (1.5-buffering): the next chunk's kf half-0 can start
                # as soon as this chunk's kv has consumed fb 0..15.
                FBH = FB // 2
                kfs_h = [ffp.tile([128, FBH, TCB], bf16, name=f"kfs{h}",
                                  tag="kfs", bufs=3) for h in range(2)]
                for fb in range(FB):
                    pskf = psKF.tile([128, TCB], f32, name="pskf", tag="kfps",
                                     bufs=4)
                    for cb in range(CB):
                        nc.tensor.matmul(
                            pskf,
                            wkf_sb[:, fb // 8, cb,
                                   (fb % 8) * 128:(fb % 8 + 1) * 128],
                            xk_t[:, cb, :], start=(cb == 0), stop=(cb == CB - 1))
                    kfb = ffp.tile([128, TCB], bf16, name="kfb", tag="kf", bufs=2)
                    nc.scalar.activation(out=kfb, in_=pskf, func=AF.Relu,
                                         bias=bkf_t[:, fb:fb + 1])
                    nc.vector.tensor_mul(kfs_h[fb // FBH][:, fb % FBH, :],
                                         kfb, kfb)

                for tt in range(NTT):
                    for chh in range(2):
                        kvp = psKV.tile([128, 512], f32, name="kvp", tag="kv",
                                        bufs=2)
                        for fb in range(FB):
                            nc.tensor.matmul(
                                kvp,
                                kfs_h[fb // FBH][:, fb % FBH,
                                                 tt * 128:(tt + 1) * 128],
                                wvf_sb[:, fb, chh * 512:(chh + 1) * 512],
                                start=(fb == 0), stop=(fb == FB - 1))
                        psr2 = psSm.tile([128, 512], f32, name="psr2", tag="sm")
                        for cb in range(CB):
                            nc.tensor.matmul(
                                psr2,
                                xr_t[:, cb, tt * 128:(tt + 1) * 128],
                                wrf_sb[:, cb, chh * 512:(chh + 1) * 512],
                                start=(cb == 0), stop=(cb == CB - 1))
                        # sigmoid via exp/ln only (same ACT table set):
                        # sig = exp(-ln(1+exp(-r)))
                        if with_b2r:
                            nc.vector.tensor_add(
                                psr2, psr2,
                                b2r_t[:, chh * 512:(chh + 1) * 512])
                        er2 = ffp.tile([128, 512], bf16, name="er2", tag="sr2")
                        nc.scalar.activation(out=er2, in_=psr2, func=AF.Exp,
                                             scale=-1.0)
                        nc.scalar.activation(out=er2, in_=er2, func=AF.Ln,
                                             bias=one_t)
                        nc.scalar.activation(out=er2, in_=er2, func=AF.Exp,
                                             scale=-1.0)
                        tmp = ffp.tile([128, 512], f32, name="tmp", tag="tmp", bufs=2)
                        nc.vector.tensor_mul(tmp, er2, kvp)
                        nc.vector.tensor_add(
                            tmp, tmp,
                            x2ts_prev[tt][:, chh * 512:(chh + 1) * 512])
                        nc.sync.dma_start(
                            out=y_d[t0 + tt * 128: t0 + (tt + 1) * 128,
                                    chh * 512:(chh + 1) * 512],
                            in_=tmp)
                xnT = xnT_nxt
                mix = mix_nxt
                nc.leave_named_scope(f"B{jc}", _sid, False)

    nc.compile()
    return nc


def get_program(with_b2r=False):
    key = ("nc", with_b2r)
    if key not in _CACHE:
        _CACHE[key] = _build_program(with_b2r)
    return _CACHE[key]


def host_inputs(inputs):
    """Host-side precompute: transposed bf16 weights + derived per-channel
    vectors. Returns the shared (per-core-identical) part of the in_map."""
    bf = ml_dtypes.bfloat16
    f32 = np.float32

    def v(name):
        return np.asarray(inputs[name], f32).reshape(-1)

    g1, b1 = v("ln1_g"), v("ln1_b")
    g2, b2 = v("ln2_g"), v("ln2_b")
    Wk = np.asarray(inputs["Wk"], f32)
    Wv = np.asarray(inputs["Wv"], f32)
    Wr = np.asarray(inputs["Wr"], f32)
    Wkf = np.asarray(inputs["Wk_ffn"], f32)
    Wrf = np.asarray(inputs["Wr_ffn"], f32)

    # The XBAR transpose writes standard channel blocking: transposed row
    # c = cb*128 + p lands at [partition p, block cb] — same layout as the
    # weight DMAs, so no permutation is needed anywhere.
    def tb(a):
        # [in_ch, cols] -> SBUF tile layout [128, in_blocks, cols]
        # (partition-contiguous so the DMA is 1 descriptor per partition)
        n, m = a.shape
        return np.ascontiguousarray(
            a.reshape(n // 128, 128, m).transpose(1, 0, 2).astype(bf))

    def col(x):  # channel panel [128, n]: entry [p, a] = x[a*128 + p]
        return np.ascontiguousarray(x.reshape(-1, 128).T)

    pars = np.concatenate([
        col(v("att_mix_k")), col(v("att_mix_v")), col(v("att_mix_r")),
        col(v("ffn_mix_k")), col(v("ffn_mix_r")),
        col(np.exp(-np.exp(v("time_decay"))).astype(f32)),
        col((Wk @ b1).astype(f32)),
        # u = time_first folded into the bias of the second exp flavor:
        # eku = exp(psk + Wk@b1 + u)
        col((Wk @ b1 + v("time_first")).astype(f32)),
        col((Wv @ b1).astype(f32)),
        col((-(Wr @ b1)).astype(f32)),
        col((Wkf @ b2).astype(f32)),
    ], axis=1).astype(f32)
    assert pars.shape == (128, NPARC)

    shared = {
        # LN gain folded in: row c of W.T scaled by g[c]
        "wkT": tb(Wk.T * g1[:, None]),
        "wvT": tb(Wv.T * g1[:, None]),
        "wrT": tb(Wr.T * g1[:, None]),
        "woT": tb(np.asarray(inputs["Wo"], f32).T),
        # wkf additionally split f-quarter-major: [128, 4, CB, FFN//4]
        "wkfT": np.ascontiguousarray(
            tb(Wkf.T * g2[:, None]).reshape(128, CB, 4, FFN // 4)
            .transpose(0, 2, 1, 3)),
        "wvfT": tb(np.asarray(inputs["Wv_ffn"], f32).T),
        "wrfT": tb(Wrf.T * g2[:, None]),
        "pars": np.ascontiguousarray(pars),
    }
    b2r = (Wrf @ b2).astype(f32)
    with_b2r = bool(np.any(b2r != 0.0))
    if with_b2r:
        shared["b2r"] = b2r
    return shared, with_b2r


def _ensure_axon_hooks():
    """The agent image's antenv lacks axon_hooks; bass_utils trace=True needs
    it. Install a shim wired to the injected libaxon_pjrt.so NTFF profiler."""
    try:
        import antenv.axon_hooks  # noqa: F401
        return
    except ImportError:
        pass
    import types
    mod = types.ModuleType("antenv.axon_hooks")
    mod._hook = None

    def set_axon_ntff_profile_hook(h):
        mod._hook = h

    def get_axon_ntff_profile_hook():
        return mod._hook

    mod.set_axon_ntff_profile_hook = set_axon_ntff_profile_hook
    mod.get_axon_ntff_profile_hook = get_axon_ntff_profile_hook
    sys.modules["antenv.axon_hooks"] = mod
    import antenv
    antenv.axon_hooks = mod
    try:
        from trn_agent_boot.trn_boot import _ntff_profile_via_ctypes
        so = "/opt/axon/libaxon_pjrt.so"
        if os.path.exists(so):
            mod._hook = _ntff_profile_via_ctypes(so)
    except Exception as e:  # pragma: no cover - degrade to no tracing
        print("ntff hook unavailable:", e)


def kernel(**inputs):
    from concourse import bass_utils

    shared, with_b2r = host_inputs(inputs)
    nc = get_program(with_b2r)
    X = np.asarray(inputs["x"], np.float32).astype(ml_dtypes.bfloat16)
    in_maps = [dict(shared, x=np.ascontiguousarray(X[b])) for b in range(NCORES)]

    trace = os.environ.get("KERNEL_TRACE", "0") == "1"
    if trace:
        _ensure_axon_hooks()
    res = bass_utils.run_bass_kernel_spmd(
        nc, in_maps, core_ids=list(range(NCORES)), trace=trace)
    kernel._last_exec_ns = res.exec_time_ns
    kernel._last_results = res
    out = np.stack([r["y"] for r in res.results], axis=0)
    return out


kernel._last_exec_ns = None
